# revision 1
# baseline (speedup 1.0000x reference)
"""Trainium2 Bass kernel for nn_MultiHeadCrossAttention (8-core SPMD).

Sharding: core = (batch, head-half); data parallel over the 4 batches,
tensor parallel over the 16 heads (8 per core). Host pre-transposes
activations/weights so the device kernel needs no on-chip transposes, and
sums the two head-half partial o-projections per batch.
"""
import sys

for p in ("/opt/trn_rl_repo", "/root/.axon_site/_ro/trn_rl_repo"):
    if p not in sys.path:
        sys.path.insert(0, p)



from contextlib import ExitStack

import concourse.bass as bass
import concourse.mybir as mybir
import concourse.tile as tile

F32R = mybir.dt.float32r
F32 = mybir.dt.float32
FP16 = mybir.dt.float16
EXP = mybir.ActivationFunctionType.Exp

DK = 64


def declare_io(nc, S, F, H):
    HD = H * DK
    nHP = H // 2
    io = {}
    io["hsT"] = nc.dram_tensor("hsT", [F, S], F32R, kind="ExternalInput").ap()
    io["htT"] = nc.dram_tensor("htT", [F, S], F32R, kind="ExternalInput").ap()
    io["wqT"] = nc.dram_tensor("wqT", [F, HD], F32R, kind="ExternalInput").ap()
    io["wkT"] = nc.dram_tensor("wkT", [F, HD], F32R, kind="ExternalInput").ap()
    io["wvT"] = nc.dram_tensor("wvT", [F, HD], F32R, kind="ExternalInput").ap()
    io["woT"] = nc.dram_tensor("woT", [HD, F], F32R, kind="ExternalInput").ap()
    io["ones"] = nc.dram_tensor("ones", [128, H * 65], F32R, kind="ExternalInput").ap()
    io["ones16"] = nc.dram_tensor("ones16", [128, H * 65], FP16, kind="ExternalInput").ap()
    io["zeros"] = nc.dram_tensor("zeros", [1, S], F32R, kind="ExternalInput").ap()
    io["outT"] = nc.dram_tensor("outT", [nHP, F, S], F32, kind="ExternalOutput").ap()
    return io


def build(ctx: ExitStack, tc: tile.TileContext, io, S, F, H):
    nc = tc.nc
    HD = H * DK
    nF = F // 128
    nTB = S // 128
    IBW = min(512, S)
    nIB = S // IBW
    nHP = H // 2
    TBW = min(512, S)
    nTB2 = S // TBW
    nJC = S // 256

    ec = ctx.enter_context
    ec(nc.allow_low_precision(reason="fp32r matmul pipeline; psum accum stays fp32"))
    consts = ec(tc.tile_pool(name="consts", bufs=1))
    vsb = ec(tc.tile_pool(name="vsb", bufs=1))
    qks = ec(tc.tile_pool(name="qks", bufs=1))
    vtsb = ec(tc.tile_pool(name="vtsb", bufs=1))
    work = ec(tc.tile_pool(name="work", bufs=3))
    rpool = ec(tc.tile_pool(name="rpool", bufs=4))
    ostage = ec(tc.tile_pool(name="ostage", bufs=3))
    hstream = ec(tc.tile_pool(name="hstream", bufs=nF + 2))
    wqkp = ec(tc.tile_pool(name="wqkp", bufs=2))
    wop = ec(tc.tile_pool(name="wop", bufs=1))
    pspool = ec(tc.tile_pool(name="pspool", bufs=2, space="PSUM"))

    ones_sb = consts.tile([1, 128], F32R, tag="ones")
    nc.gpsimd.dma_start(out=ones_sb[:], in_=io["ones"][0:1, 0:128])
    ones32 = consts.tile([1, 128], F32, tag="ones32")
    nc.vector.memset(ones32[:], 1.0)

    # fp16 HAM feeder: the PE clock gate only counts 16-bit matmul activity
    # toward un-throttling; f32r streams hold warmth but can't create it.
    warm16 = consts.tile([128, H * 65], FP16, tag="warm16")
    nc.gpsimd.dma_start(out=warm16[:], in_=io["ones16"])
    WW = min(512, H * 65)

    def ham_warm(n):
        for i in range(n):
            pw = pspool.tile([128, max(HD, IBW)], F32, tag="u1", name="pw", bufs=4)
            nc.tensor.matmul(
                pw[:, 0:WW], warm16[0:128, 0:128], warm16[:, 0:WW],
                start=True, stop=True,
            )

    ham_warm(24)

    # ---- Phase A: v projection (token-major, ones-augmented) ----
    v_sb = []
    with tc.tile_pool(name="wvpool", bufs=1) as wvp:
        wv_tiles = []
        for f in range(nF):
            t = wvp.tile([128, HD], F32R, tag=f"wv{f}", name=f"wv{f}")
            nc.scalar.dma_start(out=t[:], in_=io["wvT"][f * 128 : (f + 1) * 128, :])
            wv_tiles.append(t)

        for tbB in range(nTB2):
            ht_sl = []
            for f in range(nF):
                t = hstream.tile([128, TBW], F32R, tag="hs", name="hts")
                nc.sync.dma_start(
                    out=t[:],
                    in_=io["htT"][f * 128 : (f + 1) * 128, tbB * TBW : (tbB + 1) * TBW],
                )
                ht_sl.append(t)
            for sub in range(TBW // 128):
                tb = tbB * (TBW // 128) + sub
                vt = vsb.tile([128, H * 65], FP16, tag=f"v{tb}", name=f"v{tb}")
                nc.gpsimd.dma_start(out=vt[:], in_=io["ones16"])
                v_sb.append(vt)
                pa = pspool.tile([128, max(HD, IBW)], F32, tag="u1", name="pa", bufs=4)[:, 0:HD]
                for f in range(nF):
                    nc.tensor.matmul(
                        pa[:],
                        ht_sl[f][:, sub * 128 : (sub + 1) * 128],
                        wv_tiles[f][:],
                        start=(f == 0),
                        stop=(f == nF - 1),
                    )
                nc.vector.tensor_copy(
                    vt[:].rearrange("p (h c) -> p h c", c=65)[:, :, 0:64],
                    pa[:].rearrange("p (h c) -> p h c", c=64),
                )

    # ---- B(hp): q/k projection units for one head pair ----
    # qz0/qz1: zero-padded per-head query tiles so the scores matmul runs at
    # K=128 (full array) — K<128 matmuls don't count as PE activity for the
    # clock-gate un-throttle, which otherwise pins the whole attention phase
    # at 1.2 GHz. The zero half contributes nothing to the product.
    zbc = bass.AP(tensor=io["zeros"].tensor, offset=io["zeros"].offset,
                  ap=[[0, 64]] + io["zeros"].ap[1:])
    qz_sets, kt_sets = [], []
    for par in range(2):
        qz0 = qks.tile([128, S], F32R, tag=f"qz0_{par}", name=f"qz0_{par}")
        qz1 = qks.tile([128, S], F32R, tag=f"qz1_{par}", name=f"qz1_{par}")
        kt = qks.tile([128, S], F32R, tag=f"kt_{par}", name=f"kt_{par}")
        nc.gpsimd.dma_start(out=qz0[64:128, :], in_=zbc)
        nc.gpsimd.dma_start(out=qz1[0:64, :], in_=zbc)
        qz_sets.append((qz0, qz1))
        kt_sets.append(kt)

    def b_units(hp):
        """Callables; each emits one 8-MM projection chain + copies."""
        qz0, qz1 = qz_sets[hp % 2]
        kt = kt_sets[hp % 2]
        units = []
        for tb in range(nTB2):
            state = {}

            def pre(tb=tb, state=state, hp=hp):
                sl = []
                for f in range(nF):
                    t = hstream.tile([128, TBW], F32R, tag="hs", name="hss")
                    nc.sync.dma_start(
                        out=t[:],
                        in_=io["hsT"][
                            f * 128 : (f + 1) * 128, tb * TBW : (tb + 1) * TBW
                        ],
                    )
                    sl.append(t)
                state["sl"] = sl
                for which in ("q", "k"):
                    wt_sl = []
                    wsrc = io["wqT"] if which == "q" else io["wkT"]
                    for f in range(nF):
                        wt = wqkp.tile(
                            [128, 128], F32R, tag=f"w{which}{f}", name=f"w{which}{f}"
                        )
                        nc.scalar.dma_start(
                            out=wt[:],
                            in_=wsrc[
                                f * 128 : (f + 1) * 128, hp * 128 : (hp + 1) * 128
                            ],
                        )
                        wt_sl.append(wt)
                    state["w" + which] = wt_sl

            for which in ("q", "k"):

                def unit(tb=tb, which=which, state=state, qz0=qz0, qz1=qz1, kt=kt):
                    pq = pspool.tile(
                        [128, max(HD, IBW)], F32, tag="u1", name="pq", bufs=4
                    )[:, 0:TBW]
                    for f in range(nF):
                        nc.tensor.matmul(
                            pq[:],
                            state["w" + which][f][:],
                            state["sl"][f][:],
                            start=(f == 0),
                            stop=(f == nF - 1),
                        )
                    tbs = slice(tb * TBW, (tb + 1) * TBW)
                    if which == "k":
                        nc.vector.tensor_copy(kt[:, tbs], pq[:])
                    else:
                        nc.vector.tensor_copy(qz0[0:64, tbs], pq[0:64, :])
                        nc.vector.tensor_copy(qz1[64:128, tbs], pq[64:128, :])

                units.append({"pre": pre if which == "q" else None, "run": unit})
        return qz0, qz1, kt, units

    # ---- D(hp): o-projection partial units ----
    wo_tiles = []
    for hp in range(nHP):
        t = wop.tile([128, F], F32R, tag=f"wo{hp}", name=f"wo{hp}")
        nc.scalar.dma_start(out=t[:], in_=io["woT"][hp * 128 : (hp + 1) * 128, :])
        wo_tiles.append(t)

    def d_units(hp, valsT):
        units = []
        for mb in range(nF):
            for tb in range(nTB2):

                def unit(mb=mb, tb=tb, hp=hp, valsT=valsT):
                    po = pspool.tile(
                        [128, max(HD, IBW)], F32, tag="u1", name="po", bufs=4
                    )[:, 0:TBW]
                    nc.tensor.matmul(
                        po[:],
                        wo_tiles[hp][:, mb * 128 : (mb + 1) * 128],
                        valsT[:, tb * TBW : (tb + 1) * TBW],
                        start=True,
                        stop=True,
                    )
                    ot = ostage.tile([128, TBW], F32, tag="ot")
                    nc.vector.tensor_copy(ot[:], po[:])
                    nc.sync.dma_start(
                        out=io["outT"][
                            hp, mb * 128 : (mb + 1) * 128, tb * TBW : (tb + 1) * TBW
                        ],
                        in_=ot[:],
                    )

                units.append({"pre": None, "run": unit})
        return units

    # ---- Phase C with interleaving ----
    valsT_sb = [
        vtsb.tile([128, S], F32R, tag=f"vt{hp}", name=f"vt{hp}") for hp in range(nHP)
    ]

    qz0_cur, qz1_cur, kt_cur, bu = b_units(0)
    for u in bu:
        if u["pre"]:
            u["pre"]()
    for u in bu:
        u["run"]()

    queue = []       # interleave units (B next / D prev), FIFO
    pending = []     # deferred normalization closures

    def emit_vals(pv, pend, h):
        e, jc = pend
        for k in range(2):
            jb = 2 * jc + k
            nc.tensor.matmul(
                pv[:],
                v_sb[jb][:, h * 65 : (h + 1) * 65],
                e[:, k * IBW : (k + 1) * IBW],
                start=(jc == 0 and k == 0),
                stop=(jc == nJC - 1 and k == 1),
            )

    for hp in range(nHP):
        # enqueue next-pair projections and previous-pair o-projection
        nxt = b_units(hp + 1) if hp + 1 < nHP else None
        new_units = list(nxt[3]) if nxt else []
        if hp > 0:
            dspread = d_units(hp - 1, valsT_sb[hp - 1])
            merged, bi, di = [], 0, 0
            while bi < len(new_units) or di < len(dspread):
                if bi < len(new_units):
                    merged.append(new_units[bi])
                    bi += 1
                if di < len(dspread):
                    merged.append(dspread[di])
                    di += 1
            new_units = merged
        queue.extend(new_units)
        if queue and queue[0]["pre"]:
            queue[0]["pre"]()
            queue[0]["pre"] = None

        for ib in range(nIB):
            ibs = slice(ib * IBW, (ib + 1) * IBW)
            for sub in range(2):
                h = 2 * hp + sub
                hrows = slice(sub * DK, (sub + 1) * DK)
                pv = pspool.tile([128, max(HD, IBW)], F32, tag="u1", name="pv", bufs=4)[0:65, 0:IBW]
                pend_vals = None
                for jc in range(nJC):
                    ps = pspool.tile([128, 2 * IBW], F32, tag="big", name="ps")
                    qz = (qz0_cur, qz1_cur)[sub]
                    for k in range(2):
                        jb = 2 * jc + k
                        nc.tensor.matmul(
                            ps[:, k * IBW : (k + 1) * IBW],
                            kt_cur[:, jb * 128 : (jb + 1) * 128],
                            qz[:, ibs],
                            start=True,
                            stop=True,
                        )
                    if jc == min(2, nJC - 1):
                        while pending:
                            pending.pop(0)()
                    if queue and jc % 2 == 1:
                        u = queue.pop(0)
                        u["run"]()
                        if queue and queue[0]["pre"]:
                            queue[0]["pre"]()
                            queue[0]["pre"] = None
                    if pend_vals is not None:
                        emit_vals(pv, pend_vals, h)
                    e = work.tile([128, 2 * IBW], FP16, tag="e")
                    nc.scalar.activation(e[:], ps[:], EXP, scale=0.125)
                    pend_vals = (e, jc)
                emit_vals(pv, pend_vals, h)

                r = rpool.tile([1, IBW], F32, tag="r", name="r")
                nc.vector.reciprocal(r[:], pv[64:65, :])
                nc.vector.tensor_copy(valsT_sb[hp][hrows, ibs], pv[0:64, :])

                def norm(hp=hp, hrows=hrows, ibs=ibs, r=r):
                    pb = pspool.tile([128, max(HD, IBW)], F32, tag="u1", name="pb", bufs=4)[:, 0:IBW]
                    nc.tensor.matmul(pb[:], ones32[:], r[:], start=True, stop=True)
                    nc.vector.tensor_mul(
                        valsT_sb[hp][hrows, ibs],
                        valsT_sb[hp][hrows, ibs],
                        pb[hrows, :],
                    )

                pending.append(norm)
        if nxt:
            qz0_cur, qz1_cur, kt_cur = nxt[0], nxt[1], nxt[2]

    while pending:
        pending.pop(0)()
    while queue:
        u = queue.pop(0)
        if u["pre"]:
            u["pre"]()
        u["run"]()
    for u in d_units(nHP - 1, valsT_sb[nHP - 1]):
        u["run"]()


# ---- host orchestration ----


import ml_dtypes
import numpy as np

N_CORES = 8
B_FULL, S_FULL, F_FULL = 4, 2048, 1024
H_TOTAL = 16
H_PER_CORE = H_TOTAL // 2  # two cores (head-halves) per batch

_compiled = {}


def _get_compiled():
    if "nc" not in _compiled:
        from contextlib import ExitStack

        from concourse import bacc

        nc = bacc.Bacc(
            "TRN2", target_bir_lowering=False, debug=False, num_devices=N_CORES
        )
        io = declare_io(nc, S_FULL, F_FULL, H_PER_CORE)
        with tile.TileContext(nc) as tc:
            with ExitStack() as ctx:
                build(ctx, tc, io, S_FULL, F_FULL, H_PER_CORE)
        nc.compile()
        _compiled["nc"] = nc
    return _compiled["nc"]


def _shard_inputs(h_source, h_target, w_qk, w_v, w_o):
    """Per-core input maps. Core c -> batch c//2, head-half c%2."""
    c = np.ascontiguousarray
    in_maps = []
    for core in range(N_CORES):
        b, hh = divmod(core, 2)
        heads = range(hh * H_PER_CORE, (hh + 1) * H_PER_CORE)
        wq = np.concatenate([w_qk[h * 128 : h * 128 + 64] for h in heads], 0)
        wk = np.concatenate([w_qk[h * 128 + 64 : (h + 1) * 128] for h in heads], 0)
        wv = np.concatenate([w_v[h * 64 : (h + 1) * 64] for h in heads], 0)
        dcols = np.concatenate([np.arange(h * 64, (h + 1) * 64) for h in heads])
        in_maps.append(
            {
                "hsT": c(h_source[b].T),
                "htT": c(h_target[b].T),
                "wqT": c(wq.T),
                "wkT": c(wk.T),
                "wvT": c(wv.T),
                "woT": c(w_o[:, dcols].T),
                "ones": np.ones((128, H_PER_CORE * 65), np.float32),
                "ones16": np.ones((128, H_PER_CORE * 65), np.float16),
                "zeros": np.zeros((1, S_FULL), np.float32),
            }
        )
    return in_maps


def _run(h_source, h_target, w_qk, w_v, w_o, b_o, trace=False, trace_cores=None):
    from concourse.bass_utils import run_bass_kernel_spmd

    nc = _get_compiled()
    in_maps = _shard_inputs(
        np.asarray(h_source, np.float32),
        np.asarray(h_target, np.float32),
        np.asarray(w_qk, np.float32),
        np.asarray(w_v, np.float32),
        np.asarray(w_o, np.float32),
    )
    res = run_bass_kernel_spmd(
        nc,
        in_maps,
        core_ids=list(range(N_CORES)),
        trace=trace,
        trace_cores=trace_cores,
    )
    b_o = np.asarray(b_o, np.float32)
    out = np.empty((B_FULL, S_FULL, F_FULL), np.float32)
    for b in range(B_FULL):
        acc = res.results[2 * b]["outT"].sum(0) + res.results[2 * b + 1]["outT"].sum(0)
        out[b] = acc.T + b_o
    return out, res


def kernel(h_source, h_target, w_qk, w_v, w_o, b_o):
    out, _ = _run(h_source, h_target, w_qk, w_v, w_o, b_o)
    return out



# revision 15
# speedup vs baseline: 1.2711x; 1.2711x over previous
"""Trainium2 Bass kernel for nn_MultiHeadCrossAttention (8-core SPMD).

Sharding: core = (batch, head-half); data parallel over the 4 batches,
tensor parallel over the 16 heads (8 per core). All matmuls run in fp16
(PSUM accumulation stays fp32): 16-bit streams keep the PE HAM clock gate
at 8/8 without warm-up hacks. Scores use K=64 row tiling so the two heads
of a pair compute concurrently on the upper/lower halves of the PE array.
Exp is split between the Scalar engine (exact) and the Vector engine
(fp16 Schraudolph bit-trick, one tensor_scalar per slice). The o-projection
accumulates all four head-pair partials in PSUM on device.
"""
import sys

for p in ("/opt/trn_rl_repo", "/root/.axon_site/_ro/trn_rl_repo"):
    if p not in sys.path:
        sys.path.insert(0, p)


from contextlib import ExitStack

import concourse.bass as bass
import concourse.mybir as mybir
import concourse.tile as tile

F32 = mybir.dt.float32
FP16 = mybir.dt.float16
I16 = mybir.dt.int16
EXP = mybir.ActivationFunctionType.Exp
MULT = mybir.AluOpType.mult
ADD = mybir.AluOpType.add

DK = 64

# fp16 Schraudolph exp: bitcast_fp16(int16(round(S*x + B))) ~= exp(0.125*x)
EXP_S = 1024.0 * 1.4426950408889634 * 0.125
EXP_B = 15360.0 - 44.0
# columns (of 2048 per score chunk) computed exactly on the Scalar engine;
# the rest go through the DVE bit-trick (~3% per-element, ~25% of volume)
EXACT_COLS = 1536


def declare_io(nc, S, F, H):
    HD = H * DK  # 512
    io = {}
    io["hsT"] = nc.dram_tensor("hsT", [F, S], FP16, kind="ExternalInput").ap()
    io["htT"] = nc.dram_tensor("htT", [F, S], FP16, kind="ExternalInput").ap()
    io["wqT"] = nc.dram_tensor("wqT", [F, HD], FP16, kind="ExternalInput").ap()
    io["wkT"] = nc.dram_tensor("wkT", [F, HD], FP16, kind="ExternalInput").ap()
    io["wvT"] = nc.dram_tensor("wvT", [F, HD], FP16, kind="ExternalInput").ap()
    io["woT"] = nc.dram_tensor("woT", [HD, F], FP16, kind="ExternalInput").ap()
    io["outT"] = nc.dram_tensor("outT", [F, S], F32, kind="ExternalOutput").ap()
    return io


def build(ctx: ExitStack, tc: tile.TileContext, io, S, F, H):
    nc = tc.nc
    HD = H * DK          # 512 qk/v features per core
    nF = F // 128        # 8 feature tiles
    nHP = H // 2         # 4 head pairs
    TBW = 512            # token block width (projections)
    nTB = S // TBW       # 4
    IBW = 512            # query block width (attention)
    nIB = S // IBW       # 4
    KC = 256             # keys per score chunk (2 psum-bank-aligned halves)
    nJC = S // KC        # 8

    ec = ctx.enter_context
    ec(nc.allow_low_precision(reason="fp16 matmul pipeline; psum accum stays fp32"))
    consts = ec(tc.tile_pool(name="consts", bufs=1))
    hpool = ec(tc.tile_pool(name="hpool", bufs=1))      # hs+ht resident
    wpool = ec(tc.tile_pool(name="wpool", bufs=1))      # weights resident
    vpool = ec(tc.tile_pool(name="vpool", bufs=1))      # v|1 tiles resident
    qkpool = ec(tc.tile_pool(name="qkpool", bufs=1))    # kt/qt double set
    vtpool = ec(tc.tile_pool(name="vtpool", bufs=1))    # valsT resident
    work = ec(tc.tile_pool(name="work", bufs=3))        # e tiles
    rpool = ec(tc.tile_pool(name="rpool", bufs=2))      # recip rows
    ostage = ec(tc.tile_pool(name="ostage", bufs=3))    # o-proj staging
    scps = ec(tc.tile_pool(name="scps", bufs=1, space="PSUM"))   # scores 4 banks
    pvps = ec(tc.tile_pool(name="pvps", bufs=1, space="PSUM"))   # vals 2 banks
    prps = ec(tc.tile_pool(name="prps", bufs=2, space="PSUM"))   # proj 2 banks

    ones32 = consts.tile([1, 128], F32, tag="ones32")
    nc.vector.memset(ones32[:], 1.0)

    # ---- resident loads ----
    hs_sb, ht_sb = [], []
    for f in range(nF):
        t = hpool.tile([128, S], FP16, tag=f"hs{f}", name=f"hs{f}")
        nc.sync.dma_start(out=t[:], in_=io["hsT"][f * 128 : (f + 1) * 128, :])
        hs_sb.append(t)
        t2 = hpool.tile([128, S], FP16, tag=f"ht{f}", name=f"ht{f}")
        nc.sync.dma_start(out=t2[:], in_=io["htT"][f * 128 : (f + 1) * 128, :])
        ht_sb.append(t2)
    wq_sb, wk_sb, wv_sb = [], [], []
    for f in range(nF):
        for nm, src, dst in (("wq", "wqT", wq_sb), ("wk", "wkT", wk_sb), ("wv", "wvT", wv_sb)):
            t = wpool.tile([128, HD], FP16, tag=f"{nm}{f}", name=f"{nm}{f}")
            nc.scalar.dma_start(out=t[:], in_=io[src][f * 128 : (f + 1) * 128, :])
            dst.append(t)
    wo_sb = []
    for hp in range(nHP):
        t = wpool.tile([128, F], FP16, tag=f"wo{hp}", name=f"wo{hp}")
        nc.scalar.dma_start(out=t[:], in_=io["woT"][hp * 128 : (hp + 1) * 128, :])
        wo_sb.append(t)

    # ---- Phase A: v projection into [v|1]-interleaved token-major tiles ----
    v_sb = []
    for tb in range(S // 128):
        vt = vpool.tile([128, H * 65], FP16, tag=f"v{tb}", name=f"v{tb}")
        v_sb.append(vt)

    for tb in range(S // 128):
        nc.vector.memset(
            v_sb[tb][:].rearrange("p (h c) -> p h c", c=65)[:, :, 64:65], 1.0
        )

    def a_unit(tb):
        pa = prps.tile([128, TBW], F32, tag="pr", name="pa")[:, 0:HD]
        for f in range(nF):
            nc.tensor.matmul(
                pa[:],
                ht_sb[f][:, tb * 128 : (tb + 1) * 128],
                wv_sb[f][:, 0:HD],
                start=(f == 0),
                stop=(f == nF - 1),
            )
        nc.vector.tensor_copy(
            v_sb[tb][:].rearrange("p (h c) -> p h c", c=65)[:, :, 0:64],
            pa[:].rearrange("p (h c) -> p h c", c=64),
        )

    for tb in range(S // 128):
        a_unit(tb)

    # ---- B(hp): q/k projection units (resident weights + activations) ----
    kq_sets = []
    for par in range(2):
        kt = qkpool.tile([128, S], FP16, tag=f"kt{par}", name=f"kt{par}")
        qt = qkpool.tile([128, S], FP16, tag=f"qt{par}", name=f"qt{par}")
        kq_sets.append((kt, qt))

    def b_units(hp):
        kt, qt = kq_sets[hp % 2]
        units = []
        for tb in range(nTB):
            for which in ("q", "k"):

                def unit(tb=tb, which=which, hp=hp, kt=kt, qt=qt):
                    w_sb = wq_sb if which == "q" else wk_sb
                    dst = qt if which == "q" else kt
                    pq = prps.tile([128, TBW], F32, tag="pr", name="pq")
                    for f in range(nF):
                        nc.tensor.matmul(
                            pq[:],
                            w_sb[f][:, hp * 128 : (hp + 1) * 128],
                            hs_sb[f][:, tb * TBW : (tb + 1) * TBW],
                            start=(f == 0),
                            stop=(f == nF - 1),
                        )
                    nc.vector.tensor_copy(dst[:, tb * TBW : (tb + 1) * TBW], pq[:])

                units.append(unit)
        return kt, qt, units

    kt_cur, qt_cur, bu = b_units(0)
    for u in bu:
        u()

    # ---- Phase C: attention with interleaved next-pair projections ----
    valsT_sb = [
        vtpool.tile([128, S], FP16, tag=f"vt{hp}", name=f"vt{hp}") for hp in range(nHP)
    ]

    queue = []    # deferred projection units for the next head pair
    pending = []  # deferred normalization closures

    def emit_av(e, pv_pair, hp, jc):
        for par in range(2):
            for kb in range(2):
                jb = 2 * jc + kb
                nc.tensor.matmul(
                    pv_pair[par][:],
                    v_sb[jb][:, (2 * hp + par) * 65 : (2 * hp + par + 1) * 65],
                    e[:, par * 1024 + kb * 512 : par * 1024 + (kb + 1) * 512],
                    start=(jc == 0 and kb == 0),
                    stop=(jc == nJC - 1 and kb == 1),
                )

    for hp in range(nHP):
        nxt = b_units(hp + 1) if hp + 1 < nHP else None
        if nxt:
            queue.extend(nxt[2])

        for ib in range(nIB):
            ibs = slice(ib * IBW, (ib + 1) * IBW)
            pv_pair = [
                pvps.tile([128, IBW], F32, tag=f"pv{par}", name=f"pv{par}")[0:65, :]
                for par in range(2)
            ]
            pend = None
            for jc in range(nJC):
                sc = scps.tile([128, 2048], F32, tag="sc", name="sc")
                for kb in range(2):
                    jb = 2 * jc + kb
                    for par in range(2):
                        nc.tensor.matmul(
                            sc[:, par * 1024 + kb * 512 : par * 1024 + (kb + 1) * 512],
                            kt_cur[par * 64 : (par + 1) * 64, jb * 128 : (jb + 1) * 128],
                            qt_cur[par * 64 : (par + 1) * 64, ibs],
                            start=True,
                            stop=True,
                        )
                if pend is not None:
                    emit_av(pend[0], pv_pair, hp, pend[1])
                if queue and jc % 4 == 1:
                    queue.pop(0)()
                if pending and jc % 4 == 3:
                    pending.pop(0)()
                e = work.tile([128, 2048], FP16, tag="e")
                nc.scalar.activation(e[:, 0:EXACT_COLS], sc[:, 0:EXACT_COLS], EXP, scale=0.125)
                if EXACT_COLS < 2048:
                    nc.vector.tensor_scalar(
                        e[:].bitcast(I16)[:, EXACT_COLS:2048],
                        sc[:, EXACT_COLS:2048],
                        EXP_S,
                        EXP_B,
                        MULT,
                        ADD,
                    )
                pend = (e, jc)
            emit_av(pend[0], pv_pair, hp, pend[1])

            for par in range(2):
                h_rows = slice(par * 64, (par + 1) * 64)
                d = rpool.tile([1, IBW], F32, tag="d", name="d")
                nc.vector.tensor_copy(d[:], pv_pair[par][64:65, :])
                r = rpool.tile([1, IBW], F32, tag="r", name="r", bufs=4)
                nc.vector.reciprocal_approx_fast(out=r[:], in_=d[:])
                nc.vector.tensor_copy(valsT_sb[hp][h_rows, ibs], pv_pair[par][0:64, :])

                def norm(hp=hp, h_rows=h_rows, ibs=ibs, r=r):
                    pb = prps.tile([128, TBW], F32, tag="pr", name="pb")[:, 0:IBW]
                    nc.tensor.matmul(pb[:], ones32[:], r[:], start=True, stop=True)
                    nc.vector.tensor_mul(
                        valsT_sb[hp][h_rows, ibs],
                        valsT_sb[hp][h_rows, ibs],
                        pb[h_rows, :],
                    )

                pending.append(norm)
        if nxt:
            kt_cur, qt_cur = nxt[0], nxt[1]

    while queue:
        queue.pop(0)()
    while pending:
        pending.pop(0)()

    # ---- Phase D: o-projection, PSUM-accumulated over head pairs ----
    for mb in range(nF):
        for tb in range(nTB):
            po = prps.tile([128, TBW], F32, tag="pr", name="po")
            for hp in range(nHP):
                nc.tensor.matmul(
                    po[:],
                    wo_sb[hp][:, mb * 128 : (mb + 1) * 128],
                    valsT_sb[hp][:, tb * TBW : (tb + 1) * TBW],
                    start=(hp == 0),
                    stop=(hp == nHP - 1),
                )
            ot = ostage.tile([128, TBW], F32, tag="ot")
            nc.scalar.copy(ot[:], po[:])
            nc.sync.dma_start(
                out=io["outT"][mb * 128 : (mb + 1) * 128, tb * TBW : (tb + 1) * TBW],
                in_=ot[:],
            )


# ---- host orchestration ----


import numpy as np

N_CORES = 8
B_FULL, S_FULL, F_FULL = 4, 2048, 1024
H_TOTAL = 16
H_PER_CORE = H_TOTAL // 2

_compiled = {}


def _get_compiled():
    if "nc" not in _compiled:
        from contextlib import ExitStack

        from concourse import bacc

        nc = bacc.Bacc(
            "TRN2", target_bir_lowering=False, debug=False, num_devices=N_CORES
        )
        io = declare_io(nc, S_FULL, F_FULL, H_PER_CORE)
        with tile.TileContext(nc) as tc:
            with ExitStack() as ctx:
                build(ctx, tc, io, S_FULL, F_FULL, H_PER_CORE)
        nc.compile()
        _compiled["nc"] = nc
    return _compiled["nc"]


def _shard_inputs(h_source, h_target, w_qk, w_v, w_o):
    """Per-core input maps. Core c -> batch c//2, head-half c%2."""

    def c16(x):
        return np.ascontiguousarray(x.astype(np.float16))

    in_maps = []
    for core in range(N_CORES):
        b, hh = divmod(core, 2)
        heads = range(hh * H_PER_CORE, (hh + 1) * H_PER_CORE)
        wq = np.concatenate([w_qk[h * 128 : h * 128 + 64] for h in heads], 0)
        wk = np.concatenate([w_qk[h * 128 + 64 : (h + 1) * 128] for h in heads], 0)
        wv = np.concatenate([w_v[h * 64 : (h + 1) * 64] for h in heads], 0)
        dcols = np.concatenate([np.arange(h * 64, (h + 1) * 64) for h in heads])
        in_maps.append(
            {
                "hsT": c16(h_source[b].T),
                "htT": c16(h_target[b].T),
                "wqT": c16(wq.T),
                "wkT": c16(wk.T),
                "wvT": c16(wv.T),
                "woT": c16(w_o[:, dcols].T),
            }
        )
    return in_maps


def _run(h_source, h_target, w_qk, w_v, w_o, b_o, trace=False, trace_cores=None):
    from concourse.bass_utils import run_bass_kernel_spmd

    nc = _get_compiled()
    in_maps = _shard_inputs(
        np.asarray(h_source, np.float32),
        np.asarray(h_target, np.float32),
        np.asarray(w_qk, np.float32),
        np.asarray(w_v, np.float32),
        np.asarray(w_o, np.float32),
    )
    res = run_bass_kernel_spmd(
        nc,
        in_maps,
        core_ids=list(range(N_CORES)),
        trace=trace,
        trace_cores=trace_cores,
    )
    b_o = np.asarray(b_o, np.float32)
    out = np.empty((B_FULL, S_FULL, F_FULL), np.float32)
    for b in range(B_FULL):
        acc = res.results[2 * b]["outT"] + res.results[2 * b + 1]["outT"]
        out[b] = acc.T + b_o
    return out, res


def kernel(h_source, h_target, w_qk, w_v, w_o, b_o):
    out, _ = _run(h_source, h_target, w_qk, w_v, w_o, b_o)
    return out


# revision 21
# speedup vs baseline: 1.4227x; 1.1193x over previous
"""Trainium2 Bass kernel for nn_MultiHeadCrossAttention (8-core SPMD).

Sharding: core = (batch, head-half); data parallel over the 4 batches,
tensor parallel over the 16 heads (8 per core). All matmuls run in fp16
(PSUM accumulation stays fp32): 16-bit streams keep the PE HAM clock gate
at 8/8 without warm-up hacks. Scores use K=64 row tiling so the two heads
of a pair compute concurrently on the upper/lower halves of the PE array.
Exp is split between the Scalar engine (exact) and the Vector engine
(fp16 Schraudolph bit-trick, one tensor_scalar per slice). The o-projection
accumulates all four head-pair partials in PSUM on device.
"""
import sys

for p in ("/opt/trn_rl_repo", "/root/.axon_site/_ro/trn_rl_repo"):
    if p not in sys.path:
        sys.path.insert(0, p)


from contextlib import ExitStack

import concourse.bass as bass
import concourse.mybir as mybir
import concourse.tile as tile

F32 = mybir.dt.float32
FP16 = mybir.dt.float16
I16 = mybir.dt.int16
EXP = mybir.ActivationFunctionType.Exp
MULT = mybir.AluOpType.mult
ADD = mybir.AluOpType.add

DK = 64

# fp16 Schraudolph exp: bitcast_fp16(int16(round(S*x + B))) ~= exp(0.125*x)
EXP_S = 1024.0 * 1.4426950408889634 * 0.125
EXP_B = 15360.0 - 44.0
# columns (of 1024 per score chunk) computed exactly on the Scalar engine;
# the rest go through the DVE bit-trick (~3% per-element)
EXACT_COLS = 832


def declare_io(nc, S, F, H):
    HD = H * DK  # 512
    io = {}
    io["hsT"] = nc.dram_tensor("hsT", [F, S], FP16, kind="ExternalInput").ap()
    io["htT"] = nc.dram_tensor("htT", [F, S], FP16, kind="ExternalInput").ap()
    io["wqT"] = nc.dram_tensor("wqT", [F, HD], FP16, kind="ExternalInput").ap()
    io["wkT"] = nc.dram_tensor("wkT", [F, HD], FP16, kind="ExternalInput").ap()
    io["wvT"] = nc.dram_tensor("wvT", [F, HD], FP16, kind="ExternalInput").ap()
    io["woT"] = nc.dram_tensor("woT", [HD, F], FP16, kind="ExternalInput").ap()
    io["outT"] = nc.dram_tensor("outT", [F, S], F32, kind="ExternalOutput").ap()
    return io


def build(ctx: ExitStack, tc: tile.TileContext, io, S, F, H):
    nc = tc.nc
    HD = H * DK          # 512 qk/v features per core
    nF = F // 128        # 8 feature tiles
    nHP = H // 2         # 4 head pairs
    TBW = 512            # token block width (projections)
    nTB = S // TBW       # 4
    IBW = 512            # query block width (attention)
    nIB = S // IBW       # 4
    KC = 128             # keys per score chunk
    nJC = S // KC        # 16

    ec = ctx.enter_context
    ec(nc.allow_low_precision(reason="fp16 matmul pipeline; psum accum stays fp32"))
    consts = ec(tc.tile_pool(name="consts", bufs=1))
    hpool = ec(tc.tile_pool(name="hpool", bufs=1))      # hs+ht resident
    wpool = ec(tc.tile_pool(name="wpool", bufs=1))      # weights resident
    vpool = ec(tc.tile_pool(name="vpool", bufs=1))      # v|1 tiles resident
    qkpool = ec(tc.tile_pool(name="qkpool", bufs=1))    # kt/qt double set
    vtpool = ec(tc.tile_pool(name="vtpool", bufs=1))    # valsT resident
    work = ec(tc.tile_pool(name="work", bufs=3))        # e tiles
    rpool = ec(tc.tile_pool(name="rpool", bufs=2))      # recip rows
    ostage = ec(tc.tile_pool(name="ostage", bufs=3))    # o-proj staging
    scps = ec(tc.tile_pool(name="scps", bufs=2, space="PSUM"))   # scores 4 banks
    pvps = ec(tc.tile_pool(name="pvps", bufs=1, space="PSUM"))   # vals 2 banks
    prps = ec(tc.tile_pool(name="prps", bufs=2, space="PSUM"))   # proj 2 banks

    ones32 = consts.tile([1, 128], F32, tag="ones32")
    nc.vector.memset(ones32[:], 1.0)

    # ---- resident loads ----
    hs_sb, ht_sb = [], []
    for f in range(nF):
        t = hpool.tile([128, S], FP16, tag=f"hs{f}", name=f"hs{f}")
        nc.sync.dma_start(out=t[:], in_=io["hsT"][f * 128 : (f + 1) * 128, :])
        hs_sb.append(t)
        t2 = hpool.tile([128, S], FP16, tag=f"ht{f}", name=f"ht{f}")
        nc.sync.dma_start(out=t2[:], in_=io["htT"][f * 128 : (f + 1) * 128, :])
        ht_sb.append(t2)
    wq_sb, wk_sb, wv_sb = [], [], []
    for f in range(nF):
        for nm, src, dst in (("wq", "wqT", wq_sb), ("wk", "wkT", wk_sb), ("wv", "wvT", wv_sb)):
            t = wpool.tile([128, HD], FP16, tag=f"{nm}{f}", name=f"{nm}{f}")
            nc.scalar.dma_start(out=t[:], in_=io[src][f * 128 : (f + 1) * 128, :])
            dst.append(t)
    wo_sb = []
    for hp in range(nHP):
        t = wpool.tile([128, F], FP16, tag=f"wo{hp}", name=f"wo{hp}")
        nc.scalar.dma_start(out=t[:], in_=io["woT"][hp * 128 : (hp + 1) * 128, :])
        wo_sb.append(t)

    # ---- Phase A: v projection into [v|1]-interleaved token-major tiles ----
    v_sb = []
    for tb in range(S // 128):
        vt = vpool.tile([128, H * 65], FP16, tag=f"v{tb}", name=f"v{tb}")
        v_sb.append(vt)

    for tb in range(S // 128):
        nc.vector.memset(
            v_sb[tb][:].rearrange("p (h c) -> p h c", c=65)[:, :, 64:65], 1.0
        )

    def a_unit(tb):
        pa = prps.tile([128, TBW], F32, tag="pr", name="pa")[:, 0:HD]
        for f in range(nF):
            nc.tensor.matmul(
                pa[:],
                ht_sb[f][:, tb * 128 : (tb + 1) * 128],
                wv_sb[f][:, 0:HD],
                start=(f == 0),
                stop=(f == nF - 1),
            )
        nc.vector.tensor_copy(
            v_sb[tb][:].rearrange("p (h c) -> p h c", c=65)[:, :, 0:64],
            pa[:].rearrange("p (h c) -> p h c", c=64),
        )

    for tb in range(S // 128):
        a_unit(tb)

    # ---- B(hp): q/k projection units (resident weights + activations) ----
    kq_sets = []
    for par in range(2):
        kt = qkpool.tile([128, S], FP16, tag=f"kt{par}", name=f"kt{par}")
        qt = qkpool.tile([128, S], FP16, tag=f"qt{par}", name=f"qt{par}")
        kq_sets.append((kt, qt))

    def b_units(hp):
        kt, qt = kq_sets[hp % 2]
        units = []
        for tb in range(nTB):
            for which in ("q", "k"):

                def unit(tb=tb, which=which, hp=hp, kt=kt, qt=qt):
                    w_sb = wq_sb if which == "q" else wk_sb
                    dst = qt if which == "q" else kt
                    pq = prps.tile([128, TBW], F32, tag="pr", name="pq")
                    for f in range(nF):
                        nc.tensor.matmul(
                            pq[:],
                            w_sb[f][:, hp * 128 : (hp + 1) * 128],
                            hs_sb[f][:, tb * TBW : (tb + 1) * TBW],
                            start=(f == 0),
                            stop=(f == nF - 1),
                        )
                    nc.vector.tensor_copy(dst[:, tb * TBW : (tb + 1) * TBW], pq[:])

                units.append(unit)
        return kt, qt, units

    kt_cur, qt_cur, bu = b_units(0)
    for u in bu:
        u()

    # ---- Phase C: attention with interleaved next-pair projections ----
    valsT_sb = [
        vtpool.tile([128, S], FP16, tag=f"vt{hp}", name=f"vt{hp}") for hp in range(nHP)
    ]

    queue = []    # deferred projection units for the next head pair
    pending = []  # deferred normalization closures

    def emit_av(e, pv_pair, hp, jc):
        for par in range(2):
            nc.tensor.matmul(
                pv_pair[par][:],
                v_sb[jc][:, (2 * hp + par) * 65 : (2 * hp + par + 1) * 65],
                e[:, par * 512 : (par + 1) * 512],
                start=(jc == 0),
                stop=(jc == nJC - 1),
            )

    for hp in range(nHP):
        nxt = b_units(hp + 1) if hp + 1 < nHP else None
        if nxt:
            queue.extend(nxt[2])

        for ib in range(nIB):
            ibs = slice(ib * IBW, (ib + 1) * IBW)
            pv_pair = [
                pvps.tile([128, IBW], F32, tag=f"pv{par}", name=f"pv{par}")[0:65, :]
                for par in range(2)
            ]
            pend = None
            for jc in range(nJC):
                sc = scps.tile([128, 1024], F32, tag="sc", name="sc")
                for par in range(2):
                    nc.tensor.matmul(
                        sc[:, par * 512 : (par + 1) * 512],
                        kt_cur[par * 64 : (par + 1) * 64, jc * 128 : (jc + 1) * 128],
                        qt_cur[par * 64 : (par + 1) * 64, ibs],
                        start=True,
                        stop=True,
                    )
                if pend is not None:
                    emit_av(pend[0], pv_pair, hp, pend[1])
                if queue and jc % 8 == 1:
                    queue.pop(0)()
                if pending and jc % 8 == 5:
                    pending.pop(0)()
                e = work.tile([128, 1024], FP16, tag="e")
                nc.scalar.activation(e[:, 0:EXACT_COLS], sc[:, 0:EXACT_COLS], EXP, scale=0.125)
                if EXACT_COLS < 1024:
                    nc.vector.tensor_scalar(
                        e[:].bitcast(I16)[:, EXACT_COLS:1024],
                        sc[:, EXACT_COLS:1024],
                        EXP_S,
                        EXP_B,
                        MULT,
                        ADD,
                    )
                pend = (e, jc)
            emit_av(pend[0], pv_pair, hp, pend[1])

            for par in range(2):
                h_rows = slice(par * 64, (par + 1) * 64)
                d = rpool.tile([1, IBW], F32, tag="d", name="d")
                nc.vector.tensor_copy(d[:], pv_pair[par][64:65, :])
                r = rpool.tile([1, IBW], F32, tag="r", name="r", bufs=4)
                nc.vector.reciprocal_approx_fast(out=r[:], in_=d[:])
                nc.vector.tensor_copy(valsT_sb[hp][h_rows, ibs], pv_pair[par][0:64, :])

                def norm(hp=hp, h_rows=h_rows, ibs=ibs, r=r):
                    pb = prps.tile([128, TBW], F32, tag="pr", name="pb")[:, 0:IBW]
                    nc.tensor.matmul(pb[:], ones32[:], r[:], start=True, stop=True)
                    nc.vector.tensor_mul(
                        valsT_sb[hp][h_rows, ibs],
                        valsT_sb[hp][h_rows, ibs],
                        pb[h_rows, :],
                    )

                pending.append(norm)
        if nxt:
            kt_cur, qt_cur = nxt[0], nxt[1]

    while queue:
        queue.pop(0)()
    while pending:
        pending.pop(0)()

    # ---- Phase D: o-projection, PSUM-accumulated over head pairs ----
    for mb in range(nF):
        for tb in range(nTB):
            po = prps.tile([128, TBW], F32, tag="pr", name="po")
            for hp in range(nHP):
                nc.tensor.matmul(
                    po[:],
                    wo_sb[hp][:, mb * 128 : (mb + 1) * 128],
                    valsT_sb[hp][:, tb * TBW : (tb + 1) * TBW],
                    start=(hp == 0),
                    stop=(hp == nHP - 1),
                )
            ot = ostage.tile([128, TBW], F32, tag="ot")
            nc.vector.tensor_copy(ot[:], po[:])
            nc.sync.dma_start(
                out=io["outT"][mb * 128 : (mb + 1) * 128, tb * TBW : (tb + 1) * TBW],
                in_=ot[:],
            )


# ---- host orchestration ----


import numpy as np

N_CORES = 8
B_FULL, S_FULL, F_FULL = 4, 2048, 1024
H_TOTAL = 16
H_PER_CORE = H_TOTAL // 2

_compiled = {}


def _get_compiled():
    if "nc" not in _compiled:
        from contextlib import ExitStack

        from concourse import bacc

        nc = bacc.Bacc(
            "TRN2", target_bir_lowering=False, debug=False, num_devices=N_CORES
        )
        io = declare_io(nc, S_FULL, F_FULL, H_PER_CORE)
        with tile.TileContext(nc) as tc:
            with ExitStack() as ctx:
                build(ctx, tc, io, S_FULL, F_FULL, H_PER_CORE)
        nc.compile()
        _compiled["nc"] = nc
    return _compiled["nc"]


def _shard_inputs(h_source, h_target, w_qk, w_v, w_o):
    """Per-core input maps. Core c -> batch c//2, head-half c%2."""

    def c16(x):
        return np.ascontiguousarray(x.astype(np.float16))

    in_maps = []
    for core in range(N_CORES):
        b, hh = divmod(core, 2)
        heads = range(hh * H_PER_CORE, (hh + 1) * H_PER_CORE)
        wq = np.concatenate([w_qk[h * 128 : h * 128 + 64] for h in heads], 0)
        wk = np.concatenate([w_qk[h * 128 + 64 : (h + 1) * 128] for h in heads], 0)
        wv = np.concatenate([w_v[h * 64 : (h + 1) * 64] for h in heads], 0)
        dcols = np.concatenate([np.arange(h * 64, (h + 1) * 64) for h in heads])
        in_maps.append(
            {
                "hsT": c16(h_source[b].T),
                "htT": c16(h_target[b].T),
                "wqT": c16(wq.T),
                "wkT": c16(wk.T),
                "wvT": c16(wv.T),
                "woT": c16(w_o[:, dcols].T),
            }
        )
    return in_maps


def _run(h_source, h_target, w_qk, w_v, w_o, b_o, trace=False, trace_cores=None):
    from concourse.bass_utils import run_bass_kernel_spmd

    nc = _get_compiled()
    in_maps = _shard_inputs(
        np.asarray(h_source, np.float32),
        np.asarray(h_target, np.float32),
        np.asarray(w_qk, np.float32),
        np.asarray(w_v, np.float32),
        np.asarray(w_o, np.float32),
    )
    res = run_bass_kernel_spmd(
        nc,
        in_maps,
        core_ids=list(range(N_CORES)),
        trace=trace,
        trace_cores=trace_cores,
    )
    b_o = np.asarray(b_o, np.float32)
    out = np.empty((B_FULL, S_FULL, F_FULL), np.float32)
    for b in range(B_FULL):
        acc = res.results[2 * b]["outT"] + res.results[2 * b + 1]["outT"]
        out[b] = acc.T + b_o
    return out, res


def kernel(h_source, h_target, w_qk, w_v, w_o, b_o):
    out, _ = _run(h_source, h_target, w_qk, w_v, w_o, b_o)
    return out


# revision 32
# speedup vs baseline: 1.4658x; 1.0303x over previous
"""Trainium2 Bass kernel for nn_MultiHeadCrossAttention (8-core SPMD).

Sharding: core = (batch, head-half); data parallel over the 4 batches,
tensor parallel over the 16 heads (8 per core). All matmuls run in fp16
(PSUM accumulation stays fp32): 16-bit streams keep the PE HAM clock gate
at 8/8 without warm-up hacks. Scores use K=64 row tiling so the two heads
of a pair compute concurrently on the upper/lower halves of the PE array.
Exp is split between the Scalar engine (exact) and the Vector engine
(fp16 Schraudolph bit-trick, one tensor_scalar per slice). The o-projection
accumulates all four head-pair partials in PSUM on device.
"""
import sys

for p in ("/opt/trn_rl_repo", "/root/.axon_site/_ro/trn_rl_repo"):
    if p not in sys.path:
        sys.path.insert(0, p)


from contextlib import ExitStack

import concourse.bass as bass
import concourse.mybir as mybir
import concourse.tile as tile

F32 = mybir.dt.float32
FP16 = mybir.dt.float16
I16 = mybir.dt.int16
EXP = mybir.ActivationFunctionType.Exp
MULT = mybir.AluOpType.mult
ADD = mybir.AluOpType.add

DK = 64

# fp16 Schraudolph exp: bitcast_fp16(int16(round(S*x + B))) ~= exp(0.125*x)
EXP_S = 1024.0 * 1.4426950408889634 * 0.125
EXP_B = 15360.0 - 44.0
# columns (of 1024 per score chunk) computed exactly on the Scalar engine;
# the rest go through the DVE bit-trick (~3% per-element)
EXACT_COLS = 832


def declare_io(nc, S, F, H):
    HD = H * DK  # 512
    io = {}
    io["hsT"] = nc.dram_tensor("hsT", [F, S], FP16, kind="ExternalInput").ap()
    io["htT"] = nc.dram_tensor("htT", [F, S], FP16, kind="ExternalInput").ap()
    io["wqT"] = nc.dram_tensor("wqT", [F, HD], FP16, kind="ExternalInput").ap()
    io["wkT"] = nc.dram_tensor("wkT", [F, HD], FP16, kind="ExternalInput").ap()
    io["wvT"] = nc.dram_tensor("wvT", [F, HD], FP16, kind="ExternalInput").ap()
    io["woT"] = nc.dram_tensor("woT", [HD, F], FP16, kind="ExternalInput").ap()
    io["outT"] = nc.dram_tensor("outT", [F, S], F32, kind="ExternalOutput").ap()
    return io


def build(ctx: ExitStack, tc: tile.TileContext, io, S, F, H):
    nc = tc.nc
    HD = H * DK          # 512 qk/v features per core
    nF = F // 128        # 8 feature tiles
    nHP = H // 2         # 4 head pairs
    TBW = 512            # token block width (projections)
    nTB = S // TBW       # 4
    IBW = 512            # query block width (attention)
    nIB = S // IBW       # 4
    KC = 128             # keys per score chunk
    nJC = S // KC        # 16

    ec = ctx.enter_context
    ec(nc.allow_low_precision(reason="fp16 matmul pipeline; psum accum stays fp32"))
    consts = ec(tc.tile_pool(name="consts", bufs=1))
    hpool = ec(tc.tile_pool(name="hpool", bufs=1))      # hs+ht resident
    wpool = ec(tc.tile_pool(name="wpool", bufs=1))      # weights resident
    vpool = ec(tc.tile_pool(name="vpool", bufs=1))      # v|1 tiles resident
    qkpool = ec(tc.tile_pool(name="qkpool", bufs=1))    # kt/qt double set
    vtpool = ec(tc.tile_pool(name="vtpool", bufs=1))    # valsT resident
    work = ec(tc.tile_pool(name="work", bufs=3))        # e tiles
    rpool = ec(tc.tile_pool(name="rpool", bufs=2))      # recip rows
    ostage = ec(tc.tile_pool(name="ostage", bufs=3))    # o-proj staging
    scps = ec(tc.tile_pool(name="scps", bufs=2, space="PSUM"))   # scores 4 banks
    pvps = ec(tc.tile_pool(name="pvps", bufs=1, space="PSUM"))   # vals 2 banks
    prps = ec(tc.tile_pool(name="prps", bufs=2, space="PSUM"))   # proj 2 banks

    ones32 = consts.tile([1, 128], F32, tag="ones32")
    nc.vector.memset(ones32[:], 1.0)

    # ---- resident loads ----
    hs_sb, ht_sb = [], []
    for f in range(nF):
        t = hpool.tile([128, S], FP16, tag=f"hs{f}", name=f"hs{f}")
        nc.sync.dma_start(out=t[:], in_=io["hsT"][f * 128 : (f + 1) * 128, :])
        hs_sb.append(t)
        t2 = hpool.tile([128, S], FP16, tag=f"ht{f}", name=f"ht{f}")
        nc.sync.dma_start(out=t2[:], in_=io["htT"][f * 128 : (f + 1) * 128, :])
        ht_sb.append(t2)
    wq_sb, wk_sb, wv_sb = [], [], []
    for f in range(nF):
        for nm, src, dst in (("wq", "wqT", wq_sb), ("wk", "wkT", wk_sb), ("wv", "wvT", wv_sb)):
            t = wpool.tile([128, HD], FP16, tag=f"{nm}{f}", name=f"{nm}{f}")
            nc.scalar.dma_start(out=t[:], in_=io[src][f * 128 : (f + 1) * 128, :])
            dst.append(t)
    wo_sb = []
    for hp in range(nHP):
        t = wpool.tile([128, F], FP16, tag=f"wo{hp}", name=f"wo{hp}")
        nc.scalar.dma_start(out=t[:], in_=io["woT"][hp * 128 : (hp + 1) * 128, :])
        wo_sb.append(t)

    # ---- Phase A: v projection into [v|1]-interleaved token-major tiles ----
    v_sb = []
    for tb in range(S // 128):
        vt = vpool.tile([128, H * 65], FP16, tag=f"v{tb}", name=f"v{tb}")
        v_sb.append(vt)

    for tb in range(S // 128):
        nc.vector.memset(
            v_sb[tb][:].rearrange("p (h c) -> p h c", c=65)[:, :, 64:65], 1.0
        )

    def a_unit(tb):
        pa = prps.tile([128, TBW], F32, tag="pr", name="pa")[:, 0:HD]
        for f in range(nF):
            nc.tensor.matmul(
                pa[:],
                ht_sb[f][:, tb * 128 : (tb + 1) * 128],
                wv_sb[f][:, 0:HD],
                start=(f == 0),
                stop=(f == nF - 1),
            )
        nc.scalar.copy(
            v_sb[tb][:].rearrange("p (h c) -> p h c", c=65)[:, :, 0:64],
            pa[:].rearrange("p (h c) -> p h c", c=64),
        )

    for tb in range(S // 128):
        a_unit(tb)

    # ---- B(hp): q/k projection units (resident weights + activations) ----
    kq_sets = []
    for par in range(2):
        kt = qkpool.tile([128, S], FP16, tag=f"kt{par}", name=f"kt{par}")
        qt = qkpool.tile([128, S], FP16, tag=f"qt{par}", name=f"qt{par}")
        kq_sets.append((kt, qt))

    def b_units(hp):
        kt, qt = kq_sets[hp % 2]
        units = []
        for tb in range(nTB):
            for which in ("q", "k"):

                def unit(tb=tb, which=which, hp=hp, kt=kt, qt=qt):
                    w_sb = wq_sb if which == "q" else wk_sb
                    dst = qt if which == "q" else kt
                    pq = prps.tile([128, TBW], F32, tag="pr", name="pq")
                    for f in range(nF):
                        nc.tensor.matmul(
                            pq[:],
                            w_sb[f][:, hp * 128 : (hp + 1) * 128],
                            hs_sb[f][:, tb * TBW : (tb + 1) * TBW],
                            start=(f == 0),
                            stop=(f == nF - 1),
                        )
                    nc.vector.tensor_copy(dst[:, tb * TBW : (tb + 1) * TBW], pq[:])

                units.append(unit)
        return kt, qt, units

    kt_cur, qt_cur, bu = b_units(0)
    for u in bu:
        u()

    # ---- Phase C: attention with interleaved next-pair projections ----
    valsT_sb = [
        vtpool.tile([128, S], FP16, tag=f"vt{hp}", name=f"vt{hp}") for hp in range(nHP)
    ]

    queue = []    # deferred projection units for the next head pair
    pending = []  # deferred normalization closures

    def emit_av(e, jc, hp, pv_pair):
        for par in range(2):
            nc.tensor.matmul(
                pv_pair[par][:],
                v_sb[jc][:, (2 * hp + par) * 65 : (2 * hp + par + 1) * 65],
                e[:, par * 512 : (par + 1) * 512],
                start=(jc == 0),
                stop=(jc == nJC - 1),
            )

    for hp in range(nHP):
        nxt = b_units(hp + 1) if hp + 1 < nHP else None
        if nxt:
            queue.extend(nxt[2])

        for ib in range(nIB):
            ibs = slice(ib * IBW, (ib + 1) * IBW)
            pv_pair = [
                pvps.tile([128, IBW], F32, tag=f"pv{par}", name=f"pv{par}")[0:65, :]
                for par in range(2)
            ]
            pend = []
            for jc in range(nJC):
                sc = scps.tile([128, 1024], F32, tag="sc", name="sc")
                for par in range(2):
                    nc.tensor.matmul(
                        sc[:, par * 512 : (par + 1) * 512],
                        kt_cur[par * 64 : (par + 1) * 64, jc * 128 : (jc + 1) * 128],
                        qt_cur[par * 64 : (par + 1) * 64, ibs],
                        start=True,
                        stop=True,
                    )
                if len(pend) >= 2:
                    emit_av(*pend.pop(0), hp, pv_pair)
                if queue and jc % 8 == 1:
                    queue.pop(0)()
                if pending and jc % 8 == 5:
                    pending.pop(0)()
                e = work.tile([128, 1024], FP16, tag="e")
                nc.scalar.activation(e[:, 0:EXACT_COLS], sc[:, 0:EXACT_COLS], EXP, scale=0.125)
                if EXACT_COLS < 1024:
                    nc.vector.tensor_scalar(
                        e[:].bitcast(I16)[:, EXACT_COLS:1024],
                        sc[:, EXACT_COLS:1024],
                        EXP_S,
                        EXP_B,
                        MULT,
                        ADD,
                    )
                pend.append((e, jc))
            while pend:
                emit_av(*pend.pop(0), hp, pv_pair)

            for par in range(2):
                h_rows = slice(par * 64, (par + 1) * 64)
                d = rpool.tile([1, IBW], F32, tag="d", name="d")
                nc.vector.tensor_copy(d[:], pv_pair[par][64:65, :])
                r = rpool.tile([1, IBW], F32, tag="r", name="r", bufs=4)
                nc.vector.reciprocal_approx_fast(out=r[:], in_=d[:])
                nc.vector.tensor_copy(valsT_sb[hp][h_rows, ibs], pv_pair[par][0:64, :])

                def norm(hp=hp, h_rows=h_rows, ibs=ibs, r=r):
                    pb = prps.tile([128, TBW], F32, tag="pr", name="pb")[:, 0:IBW]
                    nc.tensor.matmul(pb[:], ones32[:], r[:], start=True, stop=True)
                    nc.vector.tensor_mul(
                        valsT_sb[hp][h_rows, ibs],
                        valsT_sb[hp][h_rows, ibs],
                        pb[h_rows, :],
                    )

                pending.append(norm)
        if nxt:
            kt_cur, qt_cur = nxt[0], nxt[1]

    while queue:
        queue.pop(0)()
    while pending:
        pending.pop(0)()

    # ---- Phase D: o-projection, PSUM-accumulated over head pairs ----
    for mb in range(nF):
        for tb in range(nTB):
            po = prps.tile([128, TBW], F32, tag="pr", name="po")
            for hp in range(nHP):
                nc.tensor.matmul(
                    po[:],
                    wo_sb[hp][:, mb * 128 : (mb + 1) * 128],
                    valsT_sb[hp][:, tb * TBW : (tb + 1) * TBW],
                    start=(hp == 0),
                    stop=(hp == nHP - 1),
                )
            ot = ostage.tile([128, TBW], F32, tag="ot")
            nc.scalar.copy(ot[:], po[:])
            nc.sync.dma_start(
                out=io["outT"][mb * 128 : (mb + 1) * 128, tb * TBW : (tb + 1) * TBW],
                in_=ot[:],
            )


# ---- host orchestration ----


import numpy as np

N_CORES = 8
B_FULL, S_FULL, F_FULL = 4, 2048, 1024
H_TOTAL = 16
H_PER_CORE = H_TOTAL // 2

_compiled = {}


def _get_compiled():
    if "nc" not in _compiled:
        from contextlib import ExitStack

        from concourse import bacc

        nc = bacc.Bacc(
            "TRN2", target_bir_lowering=False, debug=False, num_devices=N_CORES
        )
        io = declare_io(nc, S_FULL, F_FULL, H_PER_CORE)
        with tile.TileContext(nc) as tc:
            with ExitStack() as ctx:
                build(ctx, tc, io, S_FULL, F_FULL, H_PER_CORE)
        nc.compile()
        _compiled["nc"] = nc
    return _compiled["nc"]


def _shard_inputs(h_source, h_target, w_qk, w_v, w_o):
    """Per-core input maps. Core c -> batch c//2, head-half c%2."""

    def c16(x):
        return np.ascontiguousarray(x.astype(np.float16))

    in_maps = []
    for core in range(N_CORES):
        b, hh = divmod(core, 2)
        heads = range(hh * H_PER_CORE, (hh + 1) * H_PER_CORE)
        wq = np.concatenate([w_qk[h * 128 : h * 128 + 64] for h in heads], 0)
        wk = np.concatenate([w_qk[h * 128 + 64 : (h + 1) * 128] for h in heads], 0)
        wv = np.concatenate([w_v[h * 64 : (h + 1) * 64] for h in heads], 0)
        dcols = np.concatenate([np.arange(h * 64, (h + 1) * 64) for h in heads])
        in_maps.append(
            {
                "hsT": c16(h_source[b].T),
                "htT": c16(h_target[b].T),
                "wqT": c16(wq.T),
                "wkT": c16(wk.T),
                "wvT": c16(wv.T),
                "woT": c16(w_o[:, dcols].T),
            }
        )
    return in_maps


def _run(h_source, h_target, w_qk, w_v, w_o, b_o, trace=False, trace_cores=None):
    from concourse.bass_utils import run_bass_kernel_spmd

    nc = _get_compiled()
    in_maps = _shard_inputs(
        np.asarray(h_source, np.float32),
        np.asarray(h_target, np.float32),
        np.asarray(w_qk, np.float32),
        np.asarray(w_v, np.float32),
        np.asarray(w_o, np.float32),
    )
    res = run_bass_kernel_spmd(
        nc,
        in_maps,
        core_ids=list(range(N_CORES)),
        trace=trace,
        trace_cores=trace_cores,
    )
    b_o = np.asarray(b_o, np.float32)
    out = np.empty((B_FULL, S_FULL, F_FULL), np.float32)
    for b in range(B_FULL):
        acc = res.results[2 * b]["outT"] + res.results[2 * b + 1]["outT"]
        out[b] = acc.T + b_o
    return out, res


def kernel(h_source, h_target, w_qk, w_v, w_o, b_o):
    out, _ = _run(h_source, h_target, w_qk, w_v, w_o, b_o)
    return out


# revision 33
# speedup vs baseline: 1.5644x; 1.0672x over previous
"""Trainium2 Bass kernel for nn_MultiHeadCrossAttention (8-core SPMD).

Sharding: core = (batch, head-half); data parallel over the 4 batches,
tensor parallel over the 16 heads (8 per core). All matmuls run in fp16
(PSUM accumulation stays fp32): 16-bit streams keep the PE HAM clock gate
at 8/8 without warm-up hacks. Scores use K=64 row tiling so the two heads
of a pair compute concurrently on the upper/lower halves of the PE array.
Exp is split between the Scalar engine (exact) and the Vector engine
(fp16 Schraudolph bit-trick, one tensor_scalar per slice). The o-projection
accumulates all four head-pair partials in PSUM on device.
"""
import sys

for p in ("/opt/trn_rl_repo", "/root/.axon_site/_ro/trn_rl_repo"):
    if p not in sys.path:
        sys.path.insert(0, p)


from contextlib import ExitStack

import concourse.bass as bass
import concourse.mybir as mybir
import concourse.tile as tile

F32 = mybir.dt.float32
FP16 = mybir.dt.float16
I16 = mybir.dt.int16
EXP = mybir.ActivationFunctionType.Exp
MULT = mybir.AluOpType.mult
ADD = mybir.AluOpType.add

DK = 64

# fp16 Schraudolph exp: bitcast_fp16(int16(round(S*x + B))) ~= exp(0.125*x)
EXP_S = 1024.0 * 1.4426950408889634 * 0.125
EXP_B = 15360.0 - 44.0
# columns (of 1024 per score chunk) computed exactly on the Scalar engine;
# the rest go through the DVE bit-trick (~3% per-element)
EXACT_COLS = 832


def declare_io(nc, S, F, H):
    HD = H * DK  # 512
    io = {}
    io["hsT"] = nc.dram_tensor("hsT", [F, S], FP16, kind="ExternalInput").ap()
    io["htT"] = nc.dram_tensor("htT", [F, S], FP16, kind="ExternalInput").ap()
    io["wqT"] = nc.dram_tensor("wqT", [F, HD], FP16, kind="ExternalInput").ap()
    io["wkT"] = nc.dram_tensor("wkT", [F, HD], FP16, kind="ExternalInput").ap()
    io["wvT"] = nc.dram_tensor("wvT", [F, HD], FP16, kind="ExternalInput").ap()
    io["woT"] = nc.dram_tensor("woT", [HD, F], FP16, kind="ExternalInput").ap()
    io["outT"] = nc.dram_tensor("outT", [F, S], F32, kind="ExternalOutput").ap()
    return io


def build(ctx: ExitStack, tc: tile.TileContext, io, S, F, H):
    nc = tc.nc
    HD = H * DK          # 512 qk/v features per core
    nF = F // 128        # 8 feature tiles
    nHP = H // 2         # 4 head pairs
    TBW = 512            # token block width (projections)
    nTB = S // TBW       # 4
    IBW = 512            # query block width (attention)
    nIB = S // IBW       # 4
    KC = 128             # keys per score chunk
    nJC = S // KC        # 16

    ec = ctx.enter_context
    ec(nc.allow_low_precision(reason="fp16 matmul pipeline; psum accum stays fp32"))
    consts = ec(tc.tile_pool(name="consts", bufs=1))
    hpool = ec(tc.tile_pool(name="hpool", bufs=1))      # hs+ht resident
    wpool = ec(tc.tile_pool(name="wpool", bufs=1))      # weights resident
    vpool = ec(tc.tile_pool(name="vpool", bufs=1))      # v|1 tiles resident
    qkpool = ec(tc.tile_pool(name="qkpool", bufs=1))    # kt/qt double set
    vtpool = ec(tc.tile_pool(name="vtpool", bufs=1))    # valsT resident
    work = ec(tc.tile_pool(name="work", bufs=3))        # e tiles
    rpool = ec(tc.tile_pool(name="rpool", bufs=2))      # recip rows
    ostage = ec(tc.tile_pool(name="ostage", bufs=3))    # o-proj staging
    scps = ec(tc.tile_pool(name="scps", bufs=2, space="PSUM"))   # scores 4 banks
    pvps = ec(tc.tile_pool(name="pvps", bufs=1, space="PSUM"))   # vals 2 banks
    prps = ec(tc.tile_pool(name="prps", bufs=2, space="PSUM"))   # proj 2 banks

    ones32 = consts.tile([1, 128], F32, tag="ones32")
    nc.vector.memset(ones32[:], 1.0)

    # ---- resident loads ----
    hs_sb, ht_sb = [], []
    for f in range(nF):
        t = hpool.tile([128, S], FP16, tag=f"hs{f}", name=f"hs{f}")
        nc.sync.dma_start(out=t[:], in_=io["hsT"][f * 128 : (f + 1) * 128, :])
        hs_sb.append(t)
        t2 = hpool.tile([128, S], FP16, tag=f"ht{f}", name=f"ht{f}")
        nc.sync.dma_start(out=t2[:], in_=io["htT"][f * 128 : (f + 1) * 128, :])
        ht_sb.append(t2)
    wq_sb, wk_sb, wv_sb = [], [], []
    for f in range(nF):
        for nm, src, dst in (("wq", "wqT", wq_sb), ("wk", "wkT", wk_sb), ("wv", "wvT", wv_sb)):
            t = wpool.tile([128, HD], FP16, tag=f"{nm}{f}", name=f"{nm}{f}")
            nc.scalar.dma_start(out=t[:], in_=io[src][f * 128 : (f + 1) * 128, :])
            dst.append(t)
    wo_sb = []
    for hp in range(nHP):
        t = wpool.tile([128, F], FP16, tag=f"wo{hp}", name=f"wo{hp}")
        nc.scalar.dma_start(out=t[:], in_=io["woT"][hp * 128 : (hp + 1) * 128, :])
        wo_sb.append(t)

    # ---- Phase A: v projection into [v|1]-interleaved token-major tiles ----
    v_sb = []
    for tb in range(S // 128):
        vt = vpool.tile([128, H * 65], FP16, tag=f"v{tb}", name=f"v{tb}")
        v_sb.append(vt)

    for tb in range(S // 128):
        nc.vector.memset(
            v_sb[tb][:].rearrange("p (h c) -> p h c", c=65)[:, :, 64:65], 1.0
        )

    def a_unit(tb):
        pa = prps.tile([128, TBW], F32, tag="pr", name="pa")[:, 0:HD]
        for f in range(nF):
            nc.tensor.matmul(
                pa[:],
                ht_sb[f][:, tb * 128 : (tb + 1) * 128],
                wv_sb[f][:, 0:HD],
                start=(f == 0),
                stop=(f == nF - 1),
            )
        nc.scalar.copy(
            v_sb[tb][:].rearrange("p (h c) -> p h c", c=65)[:, :, 0:64],
            pa[:].rearrange("p (h c) -> p h c", c=64),
        )

    for tb in range(S // 128):
        a_unit(tb)

    # ---- B(hp): q/k projection units (resident weights + activations) ----
    kq_sets = []
    for par in range(2):
        kt = qkpool.tile([128, S], FP16, tag=f"kt{par}", name=f"kt{par}")
        qt = qkpool.tile([128, S], FP16, tag=f"qt{par}", name=f"qt{par}")
        kq_sets.append((kt, qt))

    def b_units(hp):
        kt, qt = kq_sets[hp % 2]
        units = []
        for tb in range(nTB):
            for which in ("q", "k"):

                def unit(tb=tb, which=which, hp=hp, kt=kt, qt=qt):
                    w_sb = wq_sb if which == "q" else wk_sb
                    dst = qt if which == "q" else kt
                    pq = prps.tile([128, TBW], F32, tag="pr", name="pq")
                    for f in range(nF):
                        nc.tensor.matmul(
                            pq[:],
                            w_sb[f][:, hp * 128 : (hp + 1) * 128],
                            hs_sb[f][:, tb * TBW : (tb + 1) * TBW],
                            start=(f == 0),
                            stop=(f == nF - 1),
                        )
                    nc.vector.tensor_copy(dst[:, tb * TBW : (tb + 1) * TBW], pq[:])

                units.append(unit)
        return kt, qt, units

    kt_cur, qt_cur, bu = b_units(0)
    for u in bu:
        u()

    # ---- Phase C: attention with interleaved next-pair projections ----
    valsT_sb = [
        vtpool.tile([128, S], FP16, tag=f"vt{hp}", name=f"vt{hp}") for hp in range(nHP)
    ]

    queue = []    # deferred projection units for the next head pair
    pending = []  # deferred normalization closures

    def emit_av(e, jc, hp, pv_pair):
        for par in range(2):
            nc.tensor.matmul(
                pv_pair[par][:],
                v_sb[jc][:, (2 * hp + par) * 65 : (2 * hp + par + 1) * 65],
                e[:, par * 512 : (par + 1) * 512],
                start=(jc == 0),
                stop=(jc == nJC - 1),
            )

    post = []     # deferred pv -> SBUF drains (run early next query block)

    def make_post(hp, ibs, pv_pair, par):
        h_rows = slice(par * 64, (par + 1) * 64)

        def drain():
            d = rpool.tile([1, IBW], F32, tag="d", name="d")
            nc.vector.tensor_copy(d[:], pv_pair[par][64:65, :])
            r = rpool.tile([1, IBW], F32, tag="r", name="r", bufs=4)
            nc.vector.reciprocal_approx_fast(out=r[:], in_=d[:])
            nc.vector.tensor_copy(valsT_sb[hp][h_rows, ibs], pv_pair[par][0:64, :])

            def norm():
                pb = prps.tile([128, TBW], F32, tag="pr", name="pb")[:, 0:IBW]
                nc.tensor.matmul(pb[:], ones32[:], r[:], start=True, stop=True)
                nc.vector.tensor_mul(
                    valsT_sb[hp][h_rows, ibs],
                    valsT_sb[hp][h_rows, ibs],
                    pb[h_rows, :],
                )

            pending.append(norm)

        return drain

    for hp in range(nHP):
        nxt = b_units(hp + 1) if hp + 1 < nHP else None
        if nxt:
            queue.extend(nxt[2])

        for ib in range(nIB):
            ibs = slice(ib * IBW, (ib + 1) * IBW)
            pv_pair = [
                pvps.tile([128, IBW], F32, tag=f"pv{par}", name=f"pv{par}")[0:65, :]
                for par in range(2)
            ]
            pend = []
            for jc in range(nJC):
                sc = scps.tile([128, 1024], F32, tag="sc", name="sc")
                for par in range(2):
                    nc.tensor.matmul(
                        sc[:, par * 512 : (par + 1) * 512],
                        kt_cur[par * 64 : (par + 1) * 64, jc * 128 : (jc + 1) * 128],
                        qt_cur[par * 64 : (par + 1) * 64, ibs],
                        start=True,
                        stop=True,
                    )
                if len(pend) >= 2:
                    emit_av(*pend.pop(0), hp, pv_pair)
                # slot schedule: pack PE-heavy fillers into the pipeline-fill
                # phase (jc 0-3) so the HAM activity window never sees a lull
                if jc in (0, 3) and queue:
                    queue.pop(0)()
                elif jc in (1, 2) and post:
                    post.pop(0)()
                elif jc in (5, 7) and pending:
                    pending.pop(0)()
                e = work.tile([128, 1024], FP16, tag="e")
                nc.scalar.activation(e[:, 0:EXACT_COLS], sc[:, 0:EXACT_COLS], EXP, scale=0.125)
                if EXACT_COLS < 1024:
                    nc.vector.tensor_scalar(
                        e[:].bitcast(I16)[:, EXACT_COLS:1024],
                        sc[:, EXACT_COLS:1024],
                        EXP_S,
                        EXP_B,
                        MULT,
                        ADD,
                    )
                pend.append((e, jc))
            while pend:
                emit_av(*pend.pop(0), hp, pv_pair)
            for par in range(2):
                post.append(make_post(hp, ibs, pv_pair, par))
        if nxt:
            kt_cur, qt_cur = nxt[0], nxt[1]

    while queue:
        queue.pop(0)()
    while post:
        post.pop(0)()
    while pending:
        pending.pop(0)()

    # ---- Phase D: o-projection, PSUM-accumulated over head pairs ----
    for mb in range(nF):
        for tb in range(nTB):
            po = prps.tile([128, TBW], F32, tag="pr", name="po")
            for hp in range(nHP):
                nc.tensor.matmul(
                    po[:],
                    wo_sb[hp][:, mb * 128 : (mb + 1) * 128],
                    valsT_sb[hp][:, tb * TBW : (tb + 1) * TBW],
                    start=(hp == 0),
                    stop=(hp == nHP - 1),
                )
            ot = ostage.tile([128, TBW], F32, tag="ot")
            nc.scalar.copy(ot[:], po[:])
            nc.sync.dma_start(
                out=io["outT"][mb * 128 : (mb + 1) * 128, tb * TBW : (tb + 1) * TBW],
                in_=ot[:],
            )


# ---- host orchestration ----


import numpy as np

N_CORES = 8
B_FULL, S_FULL, F_FULL = 4, 2048, 1024
H_TOTAL = 16
H_PER_CORE = H_TOTAL // 2

_compiled = {}


def _get_compiled():
    if "nc" not in _compiled:
        from contextlib import ExitStack

        from concourse import bacc

        nc = bacc.Bacc(
            "TRN2", target_bir_lowering=False, debug=False, num_devices=N_CORES
        )
        io = declare_io(nc, S_FULL, F_FULL, H_PER_CORE)
        with tile.TileContext(nc) as tc:
            with ExitStack() as ctx:
                build(ctx, tc, io, S_FULL, F_FULL, H_PER_CORE)
        nc.compile()
        _compiled["nc"] = nc
    return _compiled["nc"]


def _shard_inputs(h_source, h_target, w_qk, w_v, w_o):
    """Per-core input maps. Core c -> batch c//2, head-half c%2."""

    def c16(x):
        return np.ascontiguousarray(x.astype(np.float16))

    in_maps = []
    for core in range(N_CORES):
        b, hh = divmod(core, 2)
        heads = range(hh * H_PER_CORE, (hh + 1) * H_PER_CORE)
        wq = np.concatenate([w_qk[h * 128 : h * 128 + 64] for h in heads], 0)
        wk = np.concatenate([w_qk[h * 128 + 64 : (h + 1) * 128] for h in heads], 0)
        wv = np.concatenate([w_v[h * 64 : (h + 1) * 64] for h in heads], 0)
        dcols = np.concatenate([np.arange(h * 64, (h + 1) * 64) for h in heads])
        in_maps.append(
            {
                "hsT": c16(h_source[b].T),
                "htT": c16(h_target[b].T),
                "wqT": c16(wq.T),
                "wkT": c16(wk.T),
                "wvT": c16(wv.T),
                "woT": c16(w_o[:, dcols].T),
            }
        )
    return in_maps


def _run(h_source, h_target, w_qk, w_v, w_o, b_o, trace=False, trace_cores=None):
    from concourse.bass_utils import run_bass_kernel_spmd

    nc = _get_compiled()
    in_maps = _shard_inputs(
        np.asarray(h_source, np.float32),
        np.asarray(h_target, np.float32),
        np.asarray(w_qk, np.float32),
        np.asarray(w_v, np.float32),
        np.asarray(w_o, np.float32),
    )
    res = run_bass_kernel_spmd(
        nc,
        in_maps,
        core_ids=list(range(N_CORES)),
        trace=trace,
        trace_cores=trace_cores,
    )
    b_o = np.asarray(b_o, np.float32)
    out = np.empty((B_FULL, S_FULL, F_FULL), np.float32)
    for b in range(B_FULL):
        acc = res.results[2 * b]["outT"] + res.results[2 * b + 1]["outT"]
        out[b] = acc.T + b_o
    return out, res


def kernel(h_source, h_target, w_qk, w_v, w_o, b_o):
    out, _ = _run(h_source, h_target, w_qk, w_v, w_o, b_o)
    return out


# revision 35
# speedup vs baseline: 1.5708x; 1.0041x over previous
"""Trainium2 Bass kernel for nn_MultiHeadCrossAttention (8-core SPMD).

Sharding: core = (batch, head-half); data parallel over the 4 batches,
tensor parallel over the 16 heads (8 per core). All matmuls run in fp16
(PSUM accumulation stays fp32): 16-bit streams keep the PE HAM clock gate
at 8/8 without warm-up hacks. Scores use K=64 row tiling so the two heads
of a pair compute concurrently on the upper/lower halves of the PE array.
Exp is split between the Scalar engine (exact) and the Vector engine
(fp16 Schraudolph bit-trick, one tensor_scalar per slice). The o-projection
accumulates all four head-pair partials in PSUM on device.
"""
import sys

for p in ("/opt/trn_rl_repo", "/root/.axon_site/_ro/trn_rl_repo"):
    if p not in sys.path:
        sys.path.insert(0, p)


from contextlib import ExitStack

import concourse.bass as bass
import concourse.mybir as mybir
import concourse.tile as tile

F32 = mybir.dt.float32
FP16 = mybir.dt.float16
I16 = mybir.dt.int16
EXP = mybir.ActivationFunctionType.Exp
MULT = mybir.AluOpType.mult
ADD = mybir.AluOpType.add

DK = 64

# fp16 Schraudolph exp: bitcast_fp16(int16(round(S*x + B))) ~= exp(0.125*x)
EXP_S = 1024.0 * 1.4426950408889634 * 0.125
EXP_B = 15360.0 - 44.0
# every APPROX_PERIOD-th key chunk runs exp on the Vector engine via the
# bit-trick (~3% per weight); cutting along keys keeps any single query's
# softmax only 1/APPROX_PERIOD approximate
APPROX_PERIOD = 4


def declare_io(nc, S, F, H):
    HD = H * DK  # 512
    io = {}
    io["hsT"] = nc.dram_tensor("hsT", [F, S], FP16, kind="ExternalInput").ap()
    io["htT"] = nc.dram_tensor("htT", [F, S], FP16, kind="ExternalInput").ap()
    io["wqT"] = nc.dram_tensor("wqT", [F, HD], FP16, kind="ExternalInput").ap()
    io["wkT"] = nc.dram_tensor("wkT", [F, HD], FP16, kind="ExternalInput").ap()
    io["wvT"] = nc.dram_tensor("wvT", [F, HD], FP16, kind="ExternalInput").ap()
    io["woT"] = nc.dram_tensor("woT", [HD, F], FP16, kind="ExternalInput").ap()
    io["outT"] = nc.dram_tensor("outT", [F, S], F32, kind="ExternalOutput").ap()
    return io


def build(ctx: ExitStack, tc: tile.TileContext, io, S, F, H):
    nc = tc.nc
    HD = H * DK          # 512 qk/v features per core
    nF = F // 128        # 8 feature tiles
    nHP = H // 2         # 4 head pairs
    TBW = 512            # token block width (projections)
    nTB = S // TBW       # 4
    IBW = 512            # query block width (attention)
    nIB = S // IBW       # 4
    KC = 128             # keys per score chunk
    nJC = S // KC        # 16

    ec = ctx.enter_context
    ec(nc.allow_low_precision(reason="fp16 matmul pipeline; psum accum stays fp32"))
    consts = ec(tc.tile_pool(name="consts", bufs=1))
    hpool = ec(tc.tile_pool(name="hpool", bufs=1))      # hs+ht resident
    wpool = ec(tc.tile_pool(name="wpool", bufs=1))      # weights resident
    vpool = ec(tc.tile_pool(name="vpool", bufs=1))      # v|1 tiles resident
    qkpool = ec(tc.tile_pool(name="qkpool", bufs=1))    # kt/qt double set
    vtpool = ec(tc.tile_pool(name="vtpool", bufs=1))    # valsT resident
    work = ec(tc.tile_pool(name="work", bufs=3))        # e tiles
    rpool = ec(tc.tile_pool(name="rpool", bufs=2))      # recip rows
    ostage = ec(tc.tile_pool(name="ostage", bufs=3))    # o-proj staging
    scps = ec(tc.tile_pool(name="scps", bufs=2, space="PSUM"))   # scores 4 banks
    pvps = ec(tc.tile_pool(name="pvps", bufs=1, space="PSUM"))   # vals 2 banks
    prps = ec(tc.tile_pool(name="prps", bufs=2, space="PSUM"))   # proj 2 banks

    ones32 = consts.tile([1, 128], F32, tag="ones32")
    nc.vector.memset(ones32[:], 1.0)

    # ---- resident loads ----
    hs_sb, ht_sb = [], []
    for f in range(nF):
        t = hpool.tile([128, S], FP16, tag=f"hs{f}", name=f"hs{f}")
        nc.sync.dma_start(out=t[:], in_=io["hsT"][f * 128 : (f + 1) * 128, :])
        hs_sb.append(t)
        t2 = hpool.tile([128, S], FP16, tag=f"ht{f}", name=f"ht{f}")
        nc.sync.dma_start(out=t2[:], in_=io["htT"][f * 128 : (f + 1) * 128, :])
        ht_sb.append(t2)
    wq_sb, wk_sb, wv_sb = [], [], []
    for f in range(nF):
        for nm, src, dst in (("wq", "wqT", wq_sb), ("wk", "wkT", wk_sb), ("wv", "wvT", wv_sb)):
            t = wpool.tile([128, HD], FP16, tag=f"{nm}{f}", name=f"{nm}{f}")
            nc.scalar.dma_start(out=t[:], in_=io[src][f * 128 : (f + 1) * 128, :])
            dst.append(t)
    wo_sb = []
    for hp in range(nHP):
        t = wpool.tile([128, F], FP16, tag=f"wo{hp}", name=f"wo{hp}")
        nc.scalar.dma_start(out=t[:], in_=io["woT"][hp * 128 : (hp + 1) * 128, :])
        wo_sb.append(t)

    # ---- Phase A: v projection into [v|1]-interleaved token-major tiles ----
    v_sb = []
    for tb in range(S // 128):
        vt = vpool.tile([128, H * 65], FP16, tag=f"v{tb}", name=f"v{tb}")
        v_sb.append(vt)

    for tb in range(S // 128):
        nc.vector.memset(
            v_sb[tb][:].rearrange("p (h c) -> p h c", c=65)[:, :, 64:65], 1.0
        )

    def a_unit(tb):
        pa = prps.tile([128, TBW], F32, tag="pr", name="pa")[:, 0:HD]
        for f in range(nF):
            nc.tensor.matmul(
                pa[:],
                ht_sb[f][:, tb * 128 : (tb + 1) * 128],
                wv_sb[f][:, 0:HD],
                start=(f == 0),
                stop=(f == nF - 1),
            )
        nc.scalar.copy(
            v_sb[tb][:].rearrange("p (h c) -> p h c", c=65)[:, :, 0:64],
            pa[:].rearrange("p (h c) -> p h c", c=64),
        )

    for tb in range(S // 128):
        a_unit(tb)

    # ---- B(hp): q/k projection units (resident weights + activations) ----
    kq_sets = []
    for par in range(2):
        kt = qkpool.tile([128, S], FP16, tag=f"kt{par}", name=f"kt{par}")
        qt = qkpool.tile([128, S], FP16, tag=f"qt{par}", name=f"qt{par}")
        kq_sets.append((kt, qt))

    def b_units(hp):
        kt, qt = kq_sets[hp % 2]
        units = []
        for tb in range(nTB):
            for which in ("q", "k"):

                def unit(tb=tb, which=which, hp=hp, kt=kt, qt=qt):
                    w_sb = wq_sb if which == "q" else wk_sb
                    dst = qt if which == "q" else kt
                    pq = prps.tile([128, TBW], F32, tag="pr", name="pq")
                    for f in range(nF):
                        nc.tensor.matmul(
                            pq[:],
                            w_sb[f][:, hp * 128 : (hp + 1) * 128],
                            hs_sb[f][:, tb * TBW : (tb + 1) * TBW],
                            start=(f == 0),
                            stop=(f == nF - 1),
                        )
                    nc.vector.tensor_copy(dst[:, tb * TBW : (tb + 1) * TBW], pq[:])

                units.append(unit)
        return kt, qt, units

    kt_cur, qt_cur, bu = b_units(0)
    for u in bu:
        u()

    # ---- Phase C: attention with interleaved next-pair projections ----
    valsT_sb = [
        vtpool.tile([128, S], FP16, tag=f"vt{hp}", name=f"vt{hp}") for hp in range(nHP)
    ]

    queue = []    # deferred projection units for the next head pair
    pending = []  # deferred normalization closures

    def emit_av(e, jc, hp, pv_pair):
        for par in range(2):
            nc.tensor.matmul(
                pv_pair[par][:],
                v_sb[jc][:, (2 * hp + par) * 65 : (2 * hp + par + 1) * 65],
                e[:, par * 512 : (par + 1) * 512],
                start=(jc == 0),
                stop=(jc == nJC - 1),
            )

    post = []     # deferred pv -> SBUF drains (run early next query block)

    def make_post(hp, ibs, pv_pair, par):
        h_rows = slice(par * 64, (par + 1) * 64)

        def drain():
            d = rpool.tile([1, IBW], F32, tag="d", name="d")
            nc.vector.tensor_copy(d[:], pv_pair[par][64:65, :])
            r = rpool.tile([1, IBW], F32, tag="r", name="r", bufs=4)
            nc.vector.reciprocal_approx_fast(out=r[:], in_=d[:])
            nc.vector.tensor_copy(valsT_sb[hp][h_rows, ibs], pv_pair[par][0:64, :])

            def norm():
                pb = prps.tile([128, TBW], F32, tag="pr", name="pb")[:, 0:IBW]
                nc.tensor.matmul(pb[:], ones32[:], r[:], start=True, stop=True)
                nc.vector.tensor_mul(
                    valsT_sb[hp][h_rows, ibs],
                    valsT_sb[hp][h_rows, ibs],
                    pb[h_rows, :],
                )

            pending.append(norm)

        return drain

    for hp in range(nHP):
        nxt = b_units(hp + 1) if hp + 1 < nHP else None
        if nxt:
            queue.extend(nxt[2])

        for ib in range(nIB):
            ibs = slice(ib * IBW, (ib + 1) * IBW)
            pv_pair = [
                pvps.tile([128, IBW], F32, tag=f"pv{par}", name=f"pv{par}")[0:65, :]
                for par in range(2)
            ]
            pend = []
            for jc in range(nJC):
                sc = scps.tile([128, 1024], F32, tag="sc", name="sc")
                for par in range(2):
                    nc.tensor.matmul(
                        sc[:, par * 512 : (par + 1) * 512],
                        kt_cur[par * 64 : (par + 1) * 64, jc * 128 : (jc + 1) * 128],
                        qt_cur[par * 64 : (par + 1) * 64, ibs],
                        start=True,
                        stop=True,
                    )
                if len(pend) >= 2:
                    emit_av(*pend.pop(0), hp, pv_pair)
                # slot schedule: pack PE-heavy fillers into the pipeline-fill
                # phase (jc 0-3) so the HAM activity window never sees a lull
                if jc in (0, 3) and queue:
                    queue.pop(0)()
                elif jc in (1, 2) and post:
                    post.pop(0)()
                elif jc in (5, 7) and pending:
                    pending.pop(0)()
                e = work.tile([128, 1024], FP16, tag="e")
                if jc % APPROX_PERIOD == APPROX_PERIOD - 1:
                    nc.vector.tensor_scalar(
                        e[:].bitcast(I16)[:], sc[:], EXP_S, EXP_B, MULT, ADD
                    )
                else:
                    nc.scalar.activation(e[:], sc[:], EXP, scale=0.125)
                pend.append((e, jc))
            while pend:
                emit_av(*pend.pop(0), hp, pv_pair)
            for par in range(2):
                post.append(make_post(hp, ibs, pv_pair, par))
        if nxt:
            kt_cur, qt_cur = nxt[0], nxt[1]

    while queue:
        queue.pop(0)()
    while post:
        post.pop(0)()
    while pending:
        pending.pop(0)()

    # ---- Phase D: o-projection, PSUM-accumulated over head pairs ----
    for mb in range(nF):
        for tb in range(nTB):
            po = prps.tile([128, TBW], F32, tag="pr", name="po")
            for hp in range(nHP):
                nc.tensor.matmul(
                    po[:],
                    wo_sb[hp][:, mb * 128 : (mb + 1) * 128],
                    valsT_sb[hp][:, tb * TBW : (tb + 1) * TBW],
                    start=(hp == 0),
                    stop=(hp == nHP - 1),
                )
            ot = ostage.tile([128, TBW], F32, tag="ot")
            nc.scalar.copy(ot[:], po[:])
            nc.sync.dma_start(
                out=io["outT"][mb * 128 : (mb + 1) * 128, tb * TBW : (tb + 1) * TBW],
                in_=ot[:],
            )


# ---- host orchestration ----


import numpy as np

N_CORES = 8
B_FULL, S_FULL, F_FULL = 4, 2048, 1024
H_TOTAL = 16
H_PER_CORE = H_TOTAL // 2

_compiled = {}


def _get_compiled():
    if "nc" not in _compiled:
        from contextlib import ExitStack

        from concourse import bacc

        nc = bacc.Bacc(
            "TRN2", target_bir_lowering=False, debug=False, num_devices=N_CORES
        )
        io = declare_io(nc, S_FULL, F_FULL, H_PER_CORE)
        with tile.TileContext(nc) as tc:
            with ExitStack() as ctx:
                build(ctx, tc, io, S_FULL, F_FULL, H_PER_CORE)
        nc.compile()
        _compiled["nc"] = nc
    return _compiled["nc"]


def _shard_inputs(h_source, h_target, w_qk, w_v, w_o):
    """Per-core input maps. Core c -> batch c//2, head-half c%2."""

    def c16(x):
        return np.ascontiguousarray(x.astype(np.float16))

    in_maps = []
    for core in range(N_CORES):
        b, hh = divmod(core, 2)
        heads = range(hh * H_PER_CORE, (hh + 1) * H_PER_CORE)
        wq = np.concatenate([w_qk[h * 128 : h * 128 + 64] for h in heads], 0)
        wk = np.concatenate([w_qk[h * 128 + 64 : (h + 1) * 128] for h in heads], 0)
        wv = np.concatenate([w_v[h * 64 : (h + 1) * 64] for h in heads], 0)
        dcols = np.concatenate([np.arange(h * 64, (h + 1) * 64) for h in heads])
        in_maps.append(
            {
                "hsT": c16(h_source[b].T),
                "htT": c16(h_target[b].T),
                "wqT": c16(wq.T),
                "wkT": c16(wk.T),
                "wvT": c16(wv.T),
                "woT": c16(w_o[:, dcols].T),
            }
        )
    return in_maps


def _run(h_source, h_target, w_qk, w_v, w_o, b_o, trace=False, trace_cores=None):
    from concourse.bass_utils import run_bass_kernel_spmd

    nc = _get_compiled()
    in_maps = _shard_inputs(
        np.asarray(h_source, np.float32),
        np.asarray(h_target, np.float32),
        np.asarray(w_qk, np.float32),
        np.asarray(w_v, np.float32),
        np.asarray(w_o, np.float32),
    )
    res = run_bass_kernel_spmd(
        nc,
        in_maps,
        core_ids=list(range(N_CORES)),
        trace=trace,
        trace_cores=trace_cores,
    )
    b_o = np.asarray(b_o, np.float32)
    out = np.empty((B_FULL, S_FULL, F_FULL), np.float32)
    for b in range(B_FULL):
        acc = res.results[2 * b]["outT"] + res.results[2 * b + 1]["outT"]
        out[b] = acc.T + b_o
    return out, res


def kernel(h_source, h_target, w_qk, w_v, w_o, b_o):
    out, _ = _run(h_source, h_target, w_qk, w_v, w_o, b_o)
    return out


# revision 39
# speedup vs baseline: 1.6334x; 1.0398x over previous
"""Trainium2 Bass kernel for nn_MultiHeadCrossAttention (8-core SPMD).

Sharding: core = (batch, head-half); data parallel over the 4 batches,
tensor parallel over the 16 heads (8 per core). All matmuls run in fp16
(PSUM accumulation stays fp32): 16-bit streams keep the PE HAM clock gate
at 8/8 without warm-up hacks. Scores use K=64 row tiling so the two heads
of a pair compute concurrently on the upper/lower halves of the PE array.
Exp is split between the Scalar engine (exact) and the Vector engine
(fp16 Schraudolph bit-trick, one tensor_scalar per slice). The o-projection
accumulates all four head-pair partials in PSUM on device.
"""
import sys

for p in ("/opt/trn_rl_repo", "/root/.axon_site/_ro/trn_rl_repo"):
    if p not in sys.path:
        sys.path.insert(0, p)


from contextlib import ExitStack

import concourse.bass as bass
import concourse.mybir as mybir
import concourse.tile as tile

F32 = mybir.dt.float32
FP16 = mybir.dt.float16
I16 = mybir.dt.int16
EXP = mybir.ActivationFunctionType.Exp
MULT = mybir.AluOpType.mult
ADD = mybir.AluOpType.add

DK = 64

# fp16 Schraudolph exp: bitcast_fp16(int16(round(S*x + B))) ~= exp(0.125*x)
EXP_S = 1024.0 * 1.4426950408889634 * 0.125
EXP_B = 15360.0 - 44.0
# every APPROX_PERIOD-th key chunk runs exp on the Vector engine via the
# bit-trick (~3% per weight); cutting along keys keeps any single query's
# softmax only 1/APPROX_PERIOD approximate
APPROX_PERIOD = 4


def declare_io(nc, S, F, H):
    HD = H * DK  # 512
    io = {}
    io["hsT"] = nc.dram_tensor("hsT", [F, S], FP16, kind="ExternalInput").ap()
    io["htT"] = nc.dram_tensor("htT", [F, S], FP16, kind="ExternalInput").ap()
    io["wqT"] = nc.dram_tensor("wqT", [F, HD], FP16, kind="ExternalInput").ap()
    io["wkT"] = nc.dram_tensor("wkT", [F, HD], FP16, kind="ExternalInput").ap()
    io["wvT"] = nc.dram_tensor("wvT", [F, HD], FP16, kind="ExternalInput").ap()
    io["woT"] = nc.dram_tensor("woT", [HD, F], FP16, kind="ExternalInput").ap()
    io["outT"] = nc.dram_tensor("outT", [F, S], F32, kind="ExternalOutput").ap()
    return io


def build(ctx: ExitStack, tc: tile.TileContext, io, S, F, H):
    nc = tc.nc
    HD = H * DK          # 512 qk/v features per core
    nF = F // 128        # 8 feature tiles
    nHP = H // 2         # 4 head pairs
    TBW = 512            # token block width (projections)
    nTB = S // TBW       # 4
    IBW = 512            # query block width (attention)
    nIB = S // IBW       # 4
    KC = 128             # keys per score chunk
    nJC = S // KC        # 16

    ec = ctx.enter_context
    ec(nc.allow_low_precision(reason="fp16 matmul pipeline; psum accum stays fp32"))
    consts = ec(tc.tile_pool(name="consts", bufs=1))
    hpool = ec(tc.tile_pool(name="hpool", bufs=1))      # hs+ht resident
    wpool = ec(tc.tile_pool(name="wpool", bufs=1))      # weights resident
    vpool = ec(tc.tile_pool(name="vpool", bufs=1))      # v|1 tiles resident
    qkpool = ec(tc.tile_pool(name="qkpool", bufs=1))    # kt/qt double set
    vtpool = ec(tc.tile_pool(name="vtpool", bufs=1))    # valsT resident
    work = ec(tc.tile_pool(name="work", bufs=3))        # e tiles
    rpool = ec(tc.tile_pool(name="rpool", bufs=2))      # recip rows
    ostage = ec(tc.tile_pool(name="ostage", bufs=3))    # o-proj staging
    scps = ec(tc.tile_pool(name="scps", bufs=2, space="PSUM"))   # scores 4 banks
    pvps = ec(tc.tile_pool(name="pvps", bufs=1, space="PSUM"))   # vals 2 banks
    prps = ec(tc.tile_pool(name="prps", bufs=2, space="PSUM"))   # proj 2 banks

    ones32 = consts.tile([1, 128], F32, tag="ones32")
    nc.vector.memset(ones32[:], 1.0)

    # ---- resident loads (phase-A dependencies first) ----
    hs_sb, ht_sb = [], []
    wq_sb, wk_sb, wv_sb = [], [], []
    for f in range(nF):
        t = wpool.tile([128, HD], FP16, tag=f"wv{f}", name=f"wv{f}")
        nc.scalar.dma_start(out=t[:], in_=io["wvT"][f * 128 : (f + 1) * 128, :])
        wv_sb.append(t)
        t2 = hpool.tile([128, S], FP16, tag=f"ht{f}", name=f"ht{f}")
        nc.sync.dma_start(out=t2[:], in_=io["htT"][f * 128 : (f + 1) * 128, :])
        ht_sb.append(t2)
    for f in range(nF):
        t = hpool.tile([128, S], FP16, tag=f"hs{f}", name=f"hs{f}")
        nc.sync.dma_start(out=t[:], in_=io["hsT"][f * 128 : (f + 1) * 128, :])
        hs_sb.append(t)
        for nm, src, dst in (("wq", "wqT", wq_sb), ("wk", "wkT", wk_sb)):
            t2 = wpool.tile([128, HD], FP16, tag=f"{nm}{f}", name=f"{nm}{f}")
            nc.scalar.dma_start(out=t2[:], in_=io[src][f * 128 : (f + 1) * 128, :])
            dst.append(t2)
    wo_sb = []
    for hp in range(nHP):
        t = wpool.tile([128, F], FP16, tag=f"wo{hp}", name=f"wo{hp}")
        nc.scalar.dma_start(out=t[:], in_=io["woT"][hp * 128 : (hp + 1) * 128, :])
        wo_sb.append(t)

    # ---- Phase A: v projection into [v|1]-interleaved token-major tiles ----
    v_sb = []
    for tb in range(S // 128):
        vt = vpool.tile([128, H * 65], FP16, tag=f"v{tb}", name=f"v{tb}")
        v_sb.append(vt)

    for tb in range(S // 128):
        nc.vector.memset(
            v_sb[tb][:].rearrange("p (h c) -> p h c", c=65)[:, :, 64:65], 1.0
        )

    def a_unit(tb):
        pa = prps.tile([128, TBW], F32, tag="pr", name="pa")[:, 0:HD]
        for f in range(nF):
            nc.tensor.matmul(
                pa[:],
                ht_sb[f][:, tb * 128 : (tb + 1) * 128],
                wv_sb[f][:, 0:HD],
                start=(f == 0),
                stop=(f == nF - 1),
            )
        nc.scalar.copy(
            v_sb[tb][:].rearrange("p (h c) -> p h c", c=65)[:, :, 0:64],
            pa[:].rearrange("p (h c) -> p h c", c=64),
        )

    # ---- B(hp): q/k projection units (resident weights + activations) ----
    kq_sets = []
    for par in range(2):
        kt = qkpool.tile([128, S], FP16, tag=f"kt{par}", name=f"kt{par}")
        qt = qkpool.tile([128, S], FP16, tag=f"qt{par}", name=f"qt{par}")
        kq_sets.append((kt, qt))

    def b_units(hp):
        kt, qt = kq_sets[hp % 2]
        units = []
        for tb in range(nTB):
            for which in ("q", "k"):

                def unit(tb=tb, which=which, hp=hp, kt=kt, qt=qt):
                    w_sb = wq_sb if which == "q" else wk_sb
                    dst = qt if which == "q" else kt
                    pq = prps.tile([128, TBW], F32, tag="pr", name="pq")
                    for f in range(nF):
                        nc.tensor.matmul(
                            pq[:],
                            w_sb[f][:, hp * 128 : (hp + 1) * 128],
                            hs_sb[f][:, tb * TBW : (tb + 1) * TBW],
                            start=(f == 0),
                            stop=(f == nF - 1),
                        )
                    nc.vector.tensor_copy(dst[:, tb * TBW : (tb + 1) * TBW], pq[:])

                units.append(unit)
        return kt, qt, units

    # interleave phase A with hp0's q/k projections for weight-load hiding
    kt_cur, qt_cur, bu = b_units(0)
    bu = list(bu)
    for tb in range(S // 128):
        a_unit(tb)
        if tb % 2 == 1 and bu:
            bu.pop(0)()
    while bu:
        bu.pop(0)()

    # ---- Phase C: attention with interleaved next-pair projections ----
    valsT_sb = [
        vtpool.tile([128, S], FP16, tag=f"vt{hp}", name=f"vt{hp}") for hp in range(nHP)
    ]

    queue = []    # deferred projection units for the next head pair
    pending = []  # deferred normalization closures

    def emit_av(e, jc, hp, pv_pair):
        for par in range(2):
            nc.tensor.matmul(
                pv_pair[par][:],
                v_sb[jc][:, (2 * hp + par) * 65 : (2 * hp + par + 1) * 65],
                e[:, par * 512 : (par + 1) * 512],
                start=(jc == 0),
                stop=(jc == nJC - 1),
            )

    post = []     # deferred pv -> SBUF drains (run early next query block)

    def make_post(hp, ibs, pv_pair, par):
        h_rows = slice(par * 64, (par + 1) * 64)

        def drain():
            d = rpool.tile([1, IBW], F32, tag="d", name="d")
            nc.vector.tensor_copy(d[:], pv_pair[par][64:65, :])
            r = rpool.tile([1, IBW], F32, tag="r", name="r", bufs=4)
            nc.vector.reciprocal_approx_fast(out=r[:], in_=d[:])
            nc.vector.tensor_copy(valsT_sb[hp][h_rows, ibs], pv_pair[par][0:64, :])

            def norm():
                pb = prps.tile([128, TBW], F32, tag="pr", name="pb")[:, 0:IBW]
                nc.tensor.matmul(pb[:], ones32[:], r[:], start=True, stop=True)
                nc.vector.tensor_mul(
                    valsT_sb[hp][h_rows, ibs],
                    valsT_sb[hp][h_rows, ibs],
                    pb[h_rows, :],
                )

            pending.append(norm)

        return drain

    for hp in range(nHP):
        nxt = b_units(hp + 1) if hp + 1 < nHP else None
        if nxt:
            queue.extend(nxt[2])

        for ib in range(nIB):
            ibs = slice(ib * IBW, (ib + 1) * IBW)
            pv_pair = [
                pvps.tile([128, IBW], F32, tag=f"pv{par}", name=f"pv{par}")[0:65, :]
                for par in range(2)
            ]
            pend = []
            for jc in range(nJC):
                sc = scps.tile([128, 1024], F32, tag="sc", name="sc")
                for par in range(2):
                    nc.tensor.matmul(
                        sc[:, par * 512 : (par + 1) * 512],
                        kt_cur[par * 64 : (par + 1) * 64, jc * 128 : (jc + 1) * 128],
                        qt_cur[par * 64 : (par + 1) * 64, ibs],
                        start=True,
                        stop=True,
                    )
                if len(pend) >= 2:
                    emit_av(*pend.pop(0), hp, pv_pair)
                # slot schedule: pack PE-heavy fillers into the pipeline-fill
                # phase (jc 0-3) so the HAM activity window never sees a lull
                if jc in (0, 3) and queue:
                    queue.pop(0)()
                elif jc in (1, 2) and post:
                    post.pop(0)()
                elif jc in (5, 7) and pending:
                    pending.pop(0)()
                e = work.tile([128, 1024], FP16, tag="e")
                if jc % APPROX_PERIOD == 0:
                    nc.vector.tensor_scalar(
                        e[:].bitcast(I16)[:], sc[:], EXP_S, EXP_B, MULT, ADD
                    )
                else:
                    nc.scalar.activation(e[:], sc[:], EXP, scale=0.125)
                pend.append((e, jc))
            while pend:
                emit_av(*pend.pop(0), hp, pv_pair)
            for par in range(2):
                post.append(make_post(hp, ibs, pv_pair, par))
        if nxt:
            kt_cur, qt_cur = nxt[0], nxt[1]

    while queue:
        queue.pop(0)()
    while post:
        post.pop(0)()
    while pending:
        pending.pop(0)()

    # ---- Phase D: o-projection, PSUM-accumulated over head pairs ----
    for mb in range(nF):
        for tb in range(nTB):
            po = prps.tile([128, TBW], F32, tag="pr", name="po")
            for hp in range(nHP):
                nc.tensor.matmul(
                    po[:],
                    wo_sb[hp][:, mb * 128 : (mb + 1) * 128],
                    valsT_sb[hp][:, tb * TBW : (tb + 1) * TBW],
                    start=(hp == 0),
                    stop=(hp == nHP - 1),
                )
            ot = ostage.tile([128, TBW], F32, tag="ot")
            nc.scalar.copy(ot[:], po[:])
            nc.sync.dma_start(
                out=io["outT"][mb * 128 : (mb + 1) * 128, tb * TBW : (tb + 1) * TBW],
                in_=ot[:],
            )


# ---- host orchestration ----


import numpy as np

N_CORES = 8
B_FULL, S_FULL, F_FULL = 4, 2048, 1024
H_TOTAL = 16
H_PER_CORE = H_TOTAL // 2

_compiled = {}


def _get_compiled():
    if "nc" not in _compiled:
        from contextlib import ExitStack

        from concourse import bacc

        nc = bacc.Bacc(
            "TRN2", target_bir_lowering=False, debug=False, num_devices=N_CORES
        )
        io = declare_io(nc, S_FULL, F_FULL, H_PER_CORE)
        with tile.TileContext(nc) as tc:
            with ExitStack() as ctx:
                build(ctx, tc, io, S_FULL, F_FULL, H_PER_CORE)
        nc.compile()
        _compiled["nc"] = nc
    return _compiled["nc"]


def _shard_inputs(h_source, h_target, w_qk, w_v, w_o):
    """Per-core input maps. Core c -> batch c//2, head-half c%2."""

    def c16(x):
        return np.ascontiguousarray(x.astype(np.float16))

    in_maps = []
    for core in range(N_CORES):
        b, hh = divmod(core, 2)
        heads = range(hh * H_PER_CORE, (hh + 1) * H_PER_CORE)
        wq = np.concatenate([w_qk[h * 128 : h * 128 + 64] for h in heads], 0)
        wk = np.concatenate([w_qk[h * 128 + 64 : (h + 1) * 128] for h in heads], 0)
        wv = np.concatenate([w_v[h * 64 : (h + 1) * 64] for h in heads], 0)
        dcols = np.concatenate([np.arange(h * 64, (h + 1) * 64) for h in heads])
        in_maps.append(
            {
                "hsT": c16(h_source[b].T),
                "htT": c16(h_target[b].T),
                "wqT": c16(wq.T),
                "wkT": c16(wk.T),
                "wvT": c16(wv.T),
                "woT": c16(w_o[:, dcols].T),
            }
        )
    return in_maps


def _run(h_source, h_target, w_qk, w_v, w_o, b_o, trace=False, trace_cores=None):
    from concourse.bass_utils import run_bass_kernel_spmd

    nc = _get_compiled()
    in_maps = _shard_inputs(
        np.asarray(h_source, np.float32),
        np.asarray(h_target, np.float32),
        np.asarray(w_qk, np.float32),
        np.asarray(w_v, np.float32),
        np.asarray(w_o, np.float32),
    )
    res = run_bass_kernel_spmd(
        nc,
        in_maps,
        core_ids=list(range(N_CORES)),
        trace=trace,
        trace_cores=trace_cores,
    )
    b_o = np.asarray(b_o, np.float32)
    out = np.empty((B_FULL, S_FULL, F_FULL), np.float32)
    for b in range(B_FULL):
        acc = res.results[2 * b]["outT"] + res.results[2 * b + 1]["outT"]
        out[b] = acc.T + b_o
    return out, res


def kernel(h_source, h_target, w_qk, w_v, w_o, b_o):
    out, _ = _run(h_source, h_target, w_qk, w_v, w_o, b_o)
    return out
